# revision 1
# baseline (speedup 1.0000x reference)
"""MoE-SIREN (nn_MoE_36146444763329) Trainium2 Bass kernel.

Dense MoE: 8 SIREN experts (1->256->256->256->256->1, sin(30*) activations),
softmax gate over experts, weighted combine. B=2, N=16384 points.

Strategy: data-parallel over the 8 NeuronCores - each core computes all 8
experts for its 4096 points; no collectives. Per core:
  - gate:    z_g = gate_w * x (K=1 matmul) -> exp (ACT, per-partition bias)
             denominator via transposed ones-matmul -> DVE reciprocal
  - layer 0: x broadcast to 128 partitions (ones-matmul), then on GPSIMD:
             zb = x*a + c, k = rne(zb) via magic-add, w = zb - k
  - hidden:  K=256 PE matmuls (2 K-chunks into PSUM), range reduction in ONE
             DVE pass via the ADD_RANGE_WRAP custom op with per-partition
             bias AP (|z+b| <= ~0.77 so a single +-1 period wrap suffices),
             then sin(2*pi*w) on ACT.
  - output:  M=1 matmuls accumulate per-expert rows into an [8,512] PSUM bank
  - combine: (y + bo) * u via scalar_tensor_tensor, per-128-point transposed
             ones-matmul for numerator/denominator, multiply by reciprocal.

All angle math uses units of full turns (weights pre-scaled by omega0/2pi on
the host) so the range reduction is "wrap to [-0.5, 0.5]" and the ACT Sin
gets scale=2*pi, keeping its input inside its valid [-pi, pi] range.
"""
import numpy as np

import concourse.bass as bass
import concourse.mybir as mybir
import concourse.tile as tile
from concourse import bacc
from concourse.bass_utils import run_bass_kernel_spmd
from concourse.dve_ops import ADD_RANGE_WRAP

F32 = mybir.dt.float32
F32R = mybir.dt.float32r
AT = mybir.ActivationFunctionType
ALU = mybir.AluOpType

B, N, E, H, NLAYERS = 2, 16384, 8, 256, 4
OMEGA0 = 30.0
NCORES = 8
PTS = B * N // NCORES            # 4096 points per core
CHUNK = 1024
NCHUNK = PTS // CHUNK            # 4
SUB = 512                        # matmul moving free dim
NSUB = CHUNK // SUB              # 4 subtiles per chunk
ZGRP = 1024                      # DVE wrap granularity (2 PSUM banks)
NZG = CHUNK // ZGRP              # 2 z-groups per chunk
NHID = NLAYERS - 1               # 3 hidden layers
TWO_PI = float(2.0 * np.pi)
SC = float(OMEGA0 / (2.0 * np.pi))   # pre-scale: radians -> turns
MAGIC = float(np.float32(1.5 * 2 ** 23))
OUTCOLS = PTS // 128             # 32 columns of transposed output per core

# consts tile column layout ([128, 256] fp32)
C_A0 = 0       # 16 cols: layer0 scale  (e*2+half)
C_C0 = 16      # 16 cols: layer0 bias
C_WO = 80      # 16 cols: output weights (e*2+kc), lhsT column [128,1]
C_GB = 96      # 1 col: gate bias (partitions 0..7)
C_BO = 97      # 1 col: output bias (partitions 0..7)
C_ONES8 = 112  # 1 col: ones (partitions 0..7)
C_GW = 104     # row 0, cols 104..111: gate weights (lhsT [1,8])
C_ONES1 = 128  # row 0, cols 128..255: ones (lhsT [1,128])
C_CH = 32      # 48 cols: hidden bias ((l-1)*16 + e*2 + half)
C_WO8 = 256    # 128 cols: zero-padded output lhsT blocks [(e*2+kc)*8 + e']

_BUILD_CACHE: dict = {}


def _build(wrap_twice: bool, sin_units: int = 1, v_bufs: int = 1,
           l0_bufs: int = 3, dve_cols=(1, 3, 5, 7, 9, 11, 13, 15), z_bufs: int = 3):
    nc = bacc.Bacc("TRN2", target_bir_lowering=False, debug=False,
                   num_devices=NCORES)

    d_x = nc.dram_tensor("x", [1, PTS], F32, kind="ExternalInput")
    d_wh = nc.dram_tensor("wh", [128, NHID * 4096], F32, kind="ExternalInput")
    d_consts = nc.dram_tensor("consts", [128, 384], F32, kind="ExternalInput")
    d_out = nc.dram_tensor("out", [128, OUTCOLS], F32, kind="ExternalOutput")

    UW = CHUNK                       # unit width (one (m, half) slab)
    NB = 8 * UW                      # big-tile width: 8 units

    with tile.TileContext(nc) as tc:
        with (
            tc.tile_pool(name="cst", bufs=1) as cst_pool,
            tc.tile_pool(name="whp", bufs=1) as wh_pool,
            tc.tile_pool(name="io", bufs=1) as io_pool,
            tc.tile_pool(name="hbuf", bufs=1) as h_pool,
            tc.tile_pool(name="vbuf", bufs=1) as v_pool,
            tc.tile_pool(name="tmp", bufs=1) as tmp_pool,
            tc.tile_pool(name="zps", bufs=1, space="PSUM") as z_ps,
            tc.tile_pool(name="yps", bufs=1, space="PSUM") as y_ps,
        ):
            t_cst = cst_pool.tile([128, 384], F32, tag="consts")
            nc.sync.dma_start(t_cst[:], d_consts[:, :])
            ap_gb = t_cst[0:8, C_GB:C_GB + 1]
            ap_bo = t_cst[0:8, C_BO:C_BO + 1]
            ap_ones8 = t_cst[0:8, C_ONES8:C_ONES8 + 1]
            ap_gw = t_cst[0:1, C_GW:C_GW + 8]
            ap_ones1 = t_cst[0:1, C_ONES1:C_ONES1 + 128]

            # hidden + output weights, rounded to fp32r via casting DMA
            t_wh = []
            for l in range(NHID):
                w = wh_pool.tile([128, 4096], F32R, tag=f"wh{l}", name=f"wh{l}")
                nc.gpsimd.dma_start(w[:], d_wh[:, l * 4096:(l + 1) * 4096])
                t_wh.append(w)
            t_wo8 = wh_pool.tile([128, 128], F32R, tag="wo8", name="wo8")
            nc.gpsimd.dma_start(t_wo8[:], d_consts[:, C_WO8:C_WO8 + 128])

            t_x = io_pool.tile([1, PTS], F32, tag="x")
            nc.sync.dma_start(t_x[:], d_x[0:1, :])

            # ---- gate preamble over all points: u = exp(gw*x+gb); rsT = 1/sum
            t_u = io_pool.tile([8, PTS], F32, tag="u")
            for s in range(PTS // SUB):
                p_zg = y_ps.tile([8, SUB], F32, tag="y", name=f"zg{s}", bufs=2)
                nc.tensor.matmul(p_zg[:], ap_gw, t_x[:, s * SUB:(s + 1) * SUB],
                                 start=True, stop=True)
                nc.scalar.activation(t_u[:, s * SUB:(s + 1) * SUB], p_zg[:],
                                     AT.Exp, bias=ap_gb, scale=1.0)
            p_den = z_ps.tile([128, ZGRP], F32, tag="z", name="pden",
                              bufs=z_bufs)
            for col in range(OUTCOLS):
                nc.tensor.matmul(p_den[:, col:col + 1],
                                 t_u[:, col * 128:(col + 1) * 128],
                                 ap_ones8, start=True, stop=True)
            t_rso = tmp_pool.tile([128, 2 * OUTCOLS], F32, tag="rso")
            nc.vector.reciprocal(t_rso[:, 0:OUTCOLS], p_den[:, 0:OUTCOLS])

            # x broadcast tiles, pipelined one chunk ahead
            t_xb = {}

            def emit_xb(c):
                t = io_pool.tile([128, CHUNK], F32, tag="xb", bufs=2,
                                 name=f"xb{c}")
                for s in range(NSUB):
                    g = c * NSUB + s
                    p_xb = z_ps.tile([128, ZGRP], F32, tag="z",
                                     name=f"pxb{c}_{s}", bufs=z_bufs)
                    nc.tensor.matmul(p_xb[:, 0:SUB], ap_ones1,
                                     t_x[:, g * SUB:(g + 1) * SUB],
                                     start=True, stop=True)
                    nc.vector.tensor_copy(t[:, s * SUB:(s + 1) * SUB],
                                          p_xb[:, 0:SUB])
                t_xb[c] = t

            emit_xb(0)

            for c in range(NCHUNK):
                if c + 1 < NCHUNK:
                    emit_xb(c + 1)

                chunk_y = [y_ps.tile([8, SUB], F32, tag="y", name=f"y{c}_{s}",
                                     bufs=2)
                           for s in range(NSUB)]

                for quad in range(2):
                    # big per-parity activation slabs: unit = m*2 + half
                    t_h = {par: h_pool.tile([128, NB], F32R, tag=f"hb{par}",
                                            name=f"hb{c}_{quad}_{par}")
                           for par in range(2)}

                    # layer 0 (affine + magic-round frac), Pool + some DVE,
                    # fully decoupled via its own double-buffered tiles so it
                    # can run ahead of the hidden-layer pipeline
                    for m in range(4):
                        e = quad * 4 + m
                        for half in range(2):
                            col = e * 2 + half
                            unit = m * 2 + half
                            eng = nc.vector if col in dve_cols else nc.gpsimd
                            t_zb = tmp_pool.tile([128, UW], F32,
                                                 tag="zb", bufs=l0_bufs,
                                                 name=f"zb{c}{quad}{col}")
                            eng.tensor_scalar(
                                t_zb[:], t_xb[c][:],
                                t_cst[:, C_A0 + col:C_A0 + col + 1],
                                t_cst[:, C_C0 + col:C_C0 + col + 1],
                                ALU.mult, ALU.add)
                            t_k = tmp_pool.tile([128, UW], F32,
                                                tag="k", bufs=l0_bufs,
                                                name=f"k{c}{quad}{col}")
                            eng.tensor_scalar(t_k[:], t_zb[:],
                                              MAGIC, MAGIC,
                                              ALU.add, ALU.subtract)
                            t_v0 = tmp_pool.tile([128, UW], F32,
                                                 tag="v0", bufs=l0_bufs,
                                                 name=f"v0_{c}{quad}{col}")
                            eng.tensor_tensor(t_v0[:], t_zb[:], t_k[:],
                                              ALU.subtract)
                            nc.scalar.activation(
                                t_h[0][:, unit * UW:(unit + 1) * UW],
                                t_v0[:], AT.Sin, bias=0.0, scale=TWO_PI)

                    # hidden layers, 4-expert staggered
                    for l in range(1, NLAYERS):
                        lw = l - 1
                        rpar = (l - 1) & 1
                        wpar = l & 1
                        for m in range(4):
                            e = quad * 4 + m
                            for half in range(2):
                                unit = m * 2 + half
                                for g in range(NZG):
                                    p_z = z_ps.tile([128, ZGRP], F32, tag="z",
                                                    name=f"z{m}{half}{g}",
                                                    bufs=z_bufs)
                                    for si in range(ZGRP // SUB):
                                        s = g * (ZGRP // SUB) + si
                                        for kc in range(2):
                                            wc = ((e * 2 + kc) * 2 + half) * 128
                                            ru = m * 2 + kc
                                            nc.tensor.matmul(
                                                p_z[:, si * SUB:(si + 1) * SUB],
                                                t_wh[lw][:, wc:wc + 128],
                                                t_h[rpar][:, ru * UW + s * SUB:
                                                           ru * UW + (s + 1) * SUB],
                                                start=(kc == 0), stop=(kc == 1))
                                    chc = C_CH + lw * 16 + e * 2 + half
                                    t_v = v_pool.tile(
                                        [128, ZGRP], F32, tag="vh",
                                        name=f"vh{c}{quad}{m}{half}", bufs=3)
                                    vsl = slice(0, ZGRP)
                                    if wrap_twice:
                                        t_t2 = tmp_pool.tile([128, ZGRP], F32,
                                                             tag="wr2")
                                        nc.vector._custom_dve(
                                            ADD_RANGE_WRAP, out=t_t2[:],
                                            in0=p_z[:],
                                            s0=t_cst[:, chc:chc + 1],
                                            s1=1.0, imm2=2.0)
                                        nc.vector._custom_dve(
                                            ADD_RANGE_WRAP,
                                            out=t_v[:, vsl],
                                            in0=t_t2[:], s0=0.0,
                                            s1=0.5, imm2=1.0)
                                    else:
                                        nc.vector._custom_dve(
                                            ADD_RANGE_WRAP,
                                            out=t_v[:, vsl],
                                            in0=p_z[:],
                                            s0=t_cst[:, chc:chc + 1],
                                            s1=0.5, imm2=1.0)
                                    nc.scalar.activation(
                                        t_h[wpar][:, unit * UW + g * ZGRP:
                                                  unit * UW + (g + 1) * ZGRP],
                                        t_v[:], AT.Sin, bias=0.0,
                                        scale=TWO_PI)

                    # output layer: long accumulation group per subtile;
                    # zero-padded M=8 lhsT adds only row e per matmul
                    for s in range(NSUB):
                        p_y = chunk_y[s]
                        for m in range(4):
                            e = quad * 4 + m
                            for kc in range(2):
                                ru = m * 2 + kc
                                blk = (e * 2 + kc) * 8
                                nc.tensor.matmul(
                                    p_y[:, :],
                                    t_wo8[:, blk:blk + 8],
                                    t_h[1][:, ru * UW + s * SUB:
                                           ru * UW + (s + 1) * SUB],
                                    start=(quad == 0 and m == 0 and kc == 0),
                                    stop=(quad == 1 and m == 3 and kc == 1),
                                    skip_group_check=True)

                # ---- combine
                t_w8 = io_pool.tile([8, CHUNK], F32, tag="w8")
                for s in range(NSUB):
                    nc.vector.scalar_tensor_tensor(
                        t_w8[:, s * SUB:(s + 1) * SUB], chunk_y[s][:], ap_bo,
                        t_u[:, (c * NSUB + s) * SUB:(c * NSUB + s + 1) * SUB],
                        ALU.add, ALU.mult)
                nco = CHUNK // 128
                p_num = z_ps.tile([128, ZGRP], F32, tag="z", name=f"pnum{c}",
                                  bufs=z_bufs)
                for col in range(nco):
                    nc.tensor.matmul(p_num[:, col:col + 1],
                                     t_w8[:, col * 128:(col + 1) * 128],
                                     ap_ones8, start=True, stop=True)
                nc.vector.tensor_tensor(
                    t_rso[:, OUTCOLS + c * nco:OUTCOLS + (c + 1) * nco],
                    p_num[:, 0:nco],
                    t_rso[:, c * nco:(c + 1) * nco], ALU.mult)

            nc.sync.dma_start(d_out[:, :], t_rso[:, OUTCOLS:2 * OUTCOLS])

    nc.compile()
    return nc


LAST_RESULT = None  # BassKernelResults of the most recent run (for test.py)


def kernel(x, gate_w, gate_b, w0, b0, wh, bh, wo, bo):
    x = np.asarray(x, dtype=np.float32)
    gate_w = np.asarray(gate_w, dtype=np.float32)
    gate_b = np.asarray(gate_b, dtype=np.float32)
    w0 = np.asarray(w0, dtype=np.float32)
    b0 = np.asarray(b0, dtype=np.float32)
    wh = np.asarray(wh, dtype=np.float32)
    bh = np.asarray(bh, dtype=np.float32)
    wo = np.asarray(wo, dtype=np.float32)
    bo = np.asarray(bo, dtype=np.float32)

    # Hidden pre-activation range (in turns) decides single vs double wrap.
    # The static L1 bound is far too pessimistic; measure on the actual data
    # with a host forward pass (batched to bound memory).
    hid_bound = 0.0
    xf0 = x.reshape(-1)
    for lo in range(0, xf0.size, 8192):
        xs = xf0[lo:lo + 8192]
        h = np.sin(OMEGA0 * (w0[:, :, 0:1] * xs[None, None, :]
                             + b0[:, :, None])).astype(np.float32)
        for l in range(NHID):
            z = SC * (np.einsum('egh,eht->egt', wh[l], h,
                                dtype=np.float32)
                      + bh[l][:, :, None]).astype(np.float32)
            hid_bound = max(hid_bound, float(np.abs(z).max()))
            h = np.sin(TWO_PI * z).astype(np.float32)
    hid_bound *= 1.02  # slack for HW fp divergence
    assert hid_bound < 2.90, f"hidden range {hid_bound} too large for 2 wraps"
    wrap_twice = hid_bound >= 1.45

    # ---- host packing (fp32)
    whp = np.zeros((128, NHID * 4096), dtype=np.float32)
    for l in range(NHID):
        for e in range(E):
            for kc in range(2):
                for mc in range(2):
                    colbase = l * 4096 + ((e * 2 + kc) * 2 + mc) * 128
                    blk = (SC * wh[l, e, mc * 128:(mc + 1) * 128,
                                   kc * 128:(kc + 1) * 128]).T  # [k, m]
                    whp[:, colbase:colbase + 128] = blk

    consts = np.zeros((128, 384), dtype=np.float32)
    for e in range(E):
        for half in range(2):
            col = e * 2 + half
            consts[:, C_A0 + col] = SC * w0[e, half * 128:(half + 1) * 128, 0]
            consts[:, C_C0 + col] = SC * b0[e, half * 128:(half + 1) * 128]
    for l in range(NHID):
        for e in range(E):
            for half in range(2):
                consts[:, C_CH + l * 16 + e * 2 + half] = \
                    SC * bh[l, e, half * 128:(half + 1) * 128]
    for e in range(E):
        for kc in range(2):
            consts[:, C_WO8 + (e * 2 + kc) * 8 + e] = \
                wo[e, 0, kc * 128:(kc + 1) * 128]
    consts[0:8, C_GB] = gate_b
    consts[0:8, C_BO] = bo[:, 0]
    consts[0:8, C_ONES8] = 1.0
    consts[0, C_GW:C_GW + 8] = gate_w[:, 0]
    consts[0, C_ONES1:C_ONES1 + 128] = 1.0

    xf = x.reshape(-1)                      # [B*N]
    in_maps = []
    for j in range(NCORES):
        xc = xf[j * PTS:(j + 1) * PTS].reshape(1, PTS).copy()
        in_maps.append({"x": xc, "wh": whp, "consts": consts})

    key = wrap_twice
    if key not in _BUILD_CACHE:
        _BUILD_CACHE[key] = _build(wrap_twice)
    nc = _BUILD_CACHE[key]

    global LAST_RESULT
    LAST_RESULT = run_bass_kernel_spmd(nc, in_maps, list(range(NCORES)))
    res = LAST_RESULT.results
    parts = []
    for j in range(NCORES):
        outT = res[j]["out"]                # [128, 32]
        parts.append(outT.T.reshape(-1))    # point t = col*128 + p
    full = np.concatenate(parts).astype(np.float32)
    return full.reshape(B, N, 1)



# revision 56
# speedup vs baseline: 1.3831x; 1.3831x over previous
"""MoE-SIREN (nn_MoE_36146444763329) Trainium2 Bass kernel, v2.

Dense MoE: 8 SIREN experts (1->256->256->256->256->1, sin(30*) activations),
softmax gate over experts, weighted combine. B=2, N=16384 points.

Data-parallel over 8 NeuronCores (4096 points each), no collectives.

v2 engine plan (per chunk of 1024 points; tile = [128, 1024]):
  - PE: all matmuls (hidden K=256 via 2 fp32r chunks, output M=8 padded,
    gate, x broadcast).  Angle math in "turns" (weights pre-scaled by
    omega0/2pi on host) so wrap = round-to-nearest subtraction.
  - DVE: AFFINE_MAGIC_WRAP custom op (1 pass: ym = in*s0 + s1, out =
    ym - rne(ym) via the 1.5*2^23 magic add) for both layer-0 (from SBUF
    x-broadcast) and hidden pre-activations (from PSUM); SIN7_PROD custom
    op (deg-7 odd minimax sine in factored form, poly gain folded into the
    next layer's weights) takes a slice of the sin work.
  - ACT: bulk of the sins; tiles whose |pre-activation| stays within the
    hardware Sin table's accurate range (~3.5 rad) skip the wrap and go
    PSUM -> sin directly with a per-partition bias.
  - Pool: layer-0 wrap via 3 standard ops for most (e, half) columns.
A tiny host-side balancer picks per-tile engine assignment from these
measured per-tile ranges; the result is cached per plan.
"""
import numpy as np

import concourse.bass as bass
import concourse.mybir as mybir
import concourse.tile as tile
from concourse import bacc
from concourse.bass_utils import run_bass_kernel_spmd

import concourse.dve_ops as dve_ops
from concourse.dve_ops import DveOp
from concourse.dve_spec import (Spec, Src0, C0, C1, C2, C3, lower,
                                _spill_c3_to_src1)
from concourse.dve_uop import DveOpSpec

F32 = mybir.dt.float32
F32R = mybir.dt.float32r
F16 = mybir.dt.float16
AT = mybir.ActivationFunctionType
ALU = mybir.AluOpType

B, N, E, H, NLAYERS = 2, 16384, 8, 256, 4
OMEGA0 = 30.0
NCORES = 8
PTS = B * N // NCORES            # 4096 points per core
CHUNK = 1024
NCHUNK = PTS // CHUNK            # 4
SUB = 512                        # matmul moving free dim
NSUB = CHUNK // SUB              # 2
NHID = NLAYERS - 1               # 3 hidden layers
TWO_PI = float(2.0 * np.pi)
SC = float(OMEGA0 / (2.0 * np.pi))   # pre-scale: radians -> turns
MAGIC = float(np.float32(1.5 * 2 ** 23))
OUTCOLS = PTS // 128             # 32 columns of transposed output per core
UW = CHUNK                       # one (m, half) slab width
NB = 8 * UW                      # big-tile width: 8 units

# deg-7 odd minimax sine fits, factored p(y) = K*y*(t-R)*(t^2 + U t + V),
# t = y^2.  std: |y| <= 0.5 (post-wrap), err 2.5e-4.  wide: |y| <= 0.585
# (wrap-skipped tiles incl. slack), err 9.4e-4.  K folds into the consumer
# weights host-side.
K_STD = -56.08679675286569
R_STD = 0.24996040959546928
U_STD = -1.1395020867858219
V_STD = 0.44785087867283263
K_WIDE = -49.32072447535861
R_WIDE = 0.24994924240031763
U_WIDE = -1.2688422239794543
V_WIDE = 0.5083875825756747

SKIP_BOUND = 0.588               # turns (~3.70 rad); tile range (w/ slack)
                                 # below this may use the ACT table without a
                                 # wrap (hw table err <= ~1e-3 at the extreme)
RANGE_SLACK = 1.02               # host-measured range -> hw guard factor

# consts tile column layout ([128, 512] fp32)
C_A0 = 0       # 16 cols: layer0 scale SC*w0, col = e*2+half
C_C0 = 16      # 16 cols: layer0 bias  SC*b0
C_BH = 32      # 48 cols: hidden bias (turns)   (l-1)*16 + e*2 + half
C_BHR = 80     # 48 cols: hidden bias (radians, *2pi)
C_GB = 128     # 1 col: gate bias (partitions 0..7)
C_BO = 129     # 1 col: output bias (partitions 0..7)
C_ONES8 = 130  # 1 col: ones (partitions 0..7)
C_VSTD = 131   # 1 col: V_STD on all partitions
C_VWIDE = 132  # 1 col: V_WIDE on all partitions
C_GW = 136     # row 0, cols 136..143: gate weights (lhsT [1,8])
C_ONES1 = 144  # row 0, cols 144..271: ones (lhsT [1,128])
C_WO8 = 272    # 128 cols: zero-padded output lhsT blocks [(e*2+kc)*8 + e']
CONSTW = 512

# hidden tile classes
WRAP_ACT, WRAP_DVE, SKIP_ACT, SKIP_DVE = 0, 1, 2, 3
# layer-0 tile classes: (wrap engine, sin engine)
L0_POOL_ACT, L0_DVE_ACT, L0_POOL_DVE, L0_DVE_DVE = 0, 1, 2, 3

_BUILD_CACHE: dict = {}


def _register(name, spec, rd1):
    """Idempotently add a DveOp to the module-level registry; sha computed
    at runtime so the pin always matches this environment's lower()."""
    for o in dve_ops.OPS:
        if o.name == name:
            return o
    row = dve_ops._CUSTOM_DVE_ROW_BASE + len(dve_ops.OPS)
    assert row < 0x20
    dve_ops._SUB_OPCODE_FOR_NAME[name] = row
    shas = {}
    for ver in ("v3", "v4"):
        s = DveOpSpec(name=name, opcode=row, uops=lower(spec, ver=ver),
                      rd1_en=rd1)
        shas[ver] = s.sha(ver)
    op = DveOp(name, spec, subdim=False, uops_sha=shas)
    dve_ops.OPS.append(op)
    dve_ops.CUSTOM_DVE_SPECS[name] = spec
    return op


def _make_ops():
    _ym = Src0 * C0 + C1
    _amw = _ym - ((_ym + C2) - C2)

    def _amw_ref(in0, in1, s0, s1, imm2):
        ym = (in0.astype(np.float32) * np.float32(s0)).astype(np.float32)
        ym = (ym + np.float32(s1)).astype(np.float32)
        k = ((ym + np.float32(imm2)).astype(np.float32)
             - np.float32(imm2)).astype(np.float32)
        return ym - k

    amw = _register("AFFINE_MAGIC_WRAP_ANT", Spec(body=_amw, reference=_amw_ref),
                    False)

    _y2 = Src0 + C0
    _t = _y2 * _y2
    _sin = _spill_c3_to_src1(((_t - C1) * ((_t + C2) * _t + C3)) * _y2)

    def _sin_ref(in0, in1, s0, s1, imm2):
        ym = in0.astype(np.float32) + np.float32(s0)
        t = ym * ym
        v = np.asarray(in1, np.float32).reshape(-1, 1)
        return ((t - np.float32(s1)) * ((t + np.float32(imm2)) * t + v)) * ym

    sin7 = _register("SIN7_PROD_ANT", Spec(body=_sin, reference=_sin_ref), True)
    return amw, sin7


AMW, SIN7 = _make_ops()


def _build(plan, z_bufs=3, y_bufs=2, hl0_bufs=2, h0_bufs=2, h1_bufs=2,
           vh_bufs=4):
    """hl0_bufs=0 merges the layer-0 slab into the par-0 slab."""
    hidden_plan, l0_plan = plan
    nc = bacc.Bacc("TRN2", target_bir_lowering=False, debug=False,
                   num_devices=NCORES)

    d_x = nc.dram_tensor("x", [1, PTS], F32, kind="ExternalInput")
    d_x16 = nc.dram_tensor("x16", [1, PTS], F16, kind="ExternalInput")
    d_g16 = nc.dram_tensor("g16", [1, 16], F16, kind="ExternalInput")
    d_wh = nc.dram_tensor("wh", [128, NHID * 4096], F16, kind="ExternalInput")
    d_wo8 = nc.dram_tensor("wo8", [128, 128], F16, kind="ExternalInput")
    d_consts = nc.dram_tensor("consts", [128, CONSTW], F32,
                              kind="ExternalInput")
    d_out = nc.dram_tensor("out", [128, OUTCOLS], F32, kind="ExternalOutput")

    with tile.TileContext(nc) as tc:
        with (
            tc.tile_pool(name="cst", bufs=1) as cst_pool,
            tc.tile_pool(name="whp", bufs=1) as wh_pool,
            tc.tile_pool(name="io", bufs=1) as io_pool,
            tc.tile_pool(name="hbuf", bufs=1) as h_pool,
            tc.tile_pool(name="vbuf", bufs=1) as v_pool,
            tc.tile_pool(name="tmp", bufs=1) as tmp_pool,
            tc.tile_pool(name="zps", bufs=1, space="PSUM") as z_ps,
            tc.tile_pool(name="yps", bufs=1, space="PSUM") as y_ps,
        ):
            t_cst = cst_pool.tile([128, CONSTW], F32, tag="consts")
            nc.sync.dma_start(t_cst[:], d_consts[:, :])
            t_x16 = io_pool.tile([1, PTS], F16, tag="x16")
            nc.sync.dma_start(t_x16[:], d_x16[0:1, :])
            t_g16 = io_pool.tile([1, 16], F16, tag="g16")
            nc.sync.dma_start(t_g16[:], d_g16[0:1, :])
            ap_gb = t_cst[0:8, C_GB:C_GB + 1]
            ap_bo = t_cst[0:8, C_BO:C_BO + 1]
            ap_ones8 = t_cst[0:8, C_ONES8:C_ONES8 + 1]
            ap_gw = t_g16[0:1, 0:8]
            ap_ones1 = t_cst[0:1, C_ONES1:C_ONES1 + 128]
            ap_vstd = t_cst[:, C_VSTD:C_VSTD + 1]
            ap_vwide = t_cst[:, C_VWIDE:C_VWIDE + 1]

            # hidden + output weights in fp16 (same PE rate, half the SBUF)
            t_wh = []
            for l in range(NHID):
                w = wh_pool.tile([128, 4096], F16, tag=f"wh{l}", name=f"wh{l}")
                nc.gpsimd.dma_start(w[:], d_wh[:, l * 4096:(l + 1) * 4096])
                t_wh.append(w)
            t_wo8 = wh_pool.tile([128, 128], F16, tag="wo8", name="wo8")
            nc.gpsimd.dma_start(t_wo8[:], d_wo8[:, :])

            t_rso = tmp_pool.tile([128, 2 * OUTCOLS], F32, tag="rso")
            t_u = io_pool.tile([8, PTS], F32, tag="u")
            t_xc = {}

            def emit_gate():
                # whole-run gate, emitted inside chunk 0 so it doesn't sit at
                # the engines' queue heads before any layer-0 work
                for s in range(PTS // SUB):
                    p_zg = y_ps.tile([8, SUB], F32, tag="y", name=f"zg{s}",
                                     bufs=y_bufs)
                    nc.tensor.matmul(p_zg[:], ap_gw,
                                     t_x16[:, s * SUB:(s + 1) * SUB],
                                     start=True, stop=True)
                    nc.scalar.activation(t_u[:, s * SUB:(s + 1) * SUB],
                                         p_zg[:], AT.Exp, bias=ap_gb,
                                         scale=1.0)
                p_den = z_ps.tile([128, CHUNK], F32, tag="z", name="pden",
                                  bufs=z_bufs)
                for col in range(OUTCOLS):
                    nc.tensor.matmul(p_den[:, col:col + 1],
                                     t_u[:, col * 128:(col + 1) * 128],
                                     ap_ones8, start=True, stop=True)
                nc.vector.reciprocal(t_rso[:, 0:OUTCOLS], p_den[:, 0:OUTCOLS])

            # x broadcast tiles, pipelined one chunk ahead (PSUM -> ACT copy)
            t_xb = {}

            def emit_xb(c):
                t_xc[c] = io_pool.tile([1, CHUNK], F32, tag="xc", bufs=2,
                                       name=f"xc{c}")
                nc.sync.dma_start(t_xc[c][:],
                                  d_x[0:1, c * CHUNK:(c + 1) * CHUNK])
                t = io_pool.tile([128, CHUNK], F32, tag="xb", bufs=2,
                                 name=f"xb{c}")
                for s in range(NSUB):
                    p_xb = z_ps.tile([128, CHUNK], F32, tag="z",
                                     name=f"pxb{c}_{s}", bufs=z_bufs)
                    nc.tensor.matmul(p_xb[:, 0:SUB], ap_ones1,
                                     t_xc[c][:, s * SUB:(s + 1) * SUB],
                                     start=True, stop=True)
                    nc.scalar.activation(t[:, s * SUB:(s + 1) * SUB],
                                         p_xb[:, 0:SUB], AT.Identity,
                                         bias=0.0, scale=1.0)
                t_xb[c] = t

            emit_xb(0)

            # layer-0 software pipelining: emit quad i+1's layer-0 block
            # before quad i's hidden layers, so its wraps+sins sit AHEAD of
            # the previous quad's hidden work in every engine queue.
            slabs = {}

            def emit_l0(c, quad):
                t_h = {0: h_pool.tile([128, NB], F16, tag="hb0",
                                      bufs=h0_bufs,
                                      name=f"hb{c}_{quad}_0"),
                       1: h_pool.tile([128, NB], F16, tag="hb1",
                                      bufs=h1_bufs,
                                      name=f"hb{c}_{quad}_1")}
                t_h["l0"] = (h_pool.tile([128, NB], F16, tag="hl0",
                                         bufs=hl0_bufs,
                                         name=f"hl{c}_{quad}")
                             if hl0_bufs else t_h[0])
                slabs[(c, quad)] = t_h
                for m in range(4):
                    e = quad * 4 + m
                    for half in range(2):
                        col = e * 2 + half
                        unit = m * 2 + half
                        cls = l0_plan[col]
                        ap_a = t_cst[:, C_A0 + col:C_A0 + col + 1]
                        ap_c = t_cst[:, C_C0 + col:C_C0 + col + 1]
                        hsl = t_h["l0"][:, unit * UW:(unit + 1) * UW]
                        # wrap written straight into the fp16 slab slot, then
                        # sin in-place: the slab double-buffering is the
                        # producer runahead
                        if cls in (L0_POOL_ACT, L0_POOL_DVE):
                            t_zb = tmp_pool.tile([128, UW], F32, tag="zb",
                                                 bufs=2,
                                                 name=f"zb{c}{quad}{col}")
                            nc.gpsimd.tensor_scalar(
                                t_zb[:], t_xb[c][:], ap_a, ap_c,
                                ALU.mult, ALU.add)
                            t_k = tmp_pool.tile([128, UW], F32, tag="k",
                                                bufs=2,
                                                name=f"k{c}{quad}{col}")
                            nc.gpsimd.tensor_scalar(
                                t_k[:], t_zb[:], MAGIC, MAGIC,
                                ALU.add, ALU.subtract)
                            nc.gpsimd.tensor_tensor(
                                hsl, t_zb[:], t_k[:], ALU.subtract)
                        else:
                            nc.vector._custom_dve(
                                AMW, out=hsl, in0=t_xb[c][:],
                                s0=ap_a, s1=ap_c, imm2=MAGIC)
                        if cls in (L0_POOL_ACT, L0_DVE_ACT):
                            nc.scalar.activation(hsl, hsl, AT.Sin,
                                                 bias=0.0, scale=TWO_PI)
                        else:
                            nc.vector._custom_dve(
                                SIN7, out=hsl, in0=hsl, in1=ap_vstd,
                                s0=0.0, s1=R_STD, imm2=U_STD)

            for c in range(NCHUNK):
                if c + 1 < NCHUNK:
                    emit_xb(c + 1)
                if c == 0:
                    emit_gate()

                chunk_y = [y_ps.tile([8, SUB], F32, tag="y", name=f"y{c}_{s}",
                                     bufs=y_bufs)
                           for s in range(NSUB)]

                for quad in range(2):
                    emit_l0(c, quad)
                    t_h = slabs.pop((c, quad))

                    # ---- hidden layers, 4-expert staggered
                    for l in range(1, NLAYERS):
                        lw = l - 1
                        rpar = "l0" if l == 1 else (l - 1) & 1
                        wpar = l & 1
                        for m in range(4):
                            e = quad * 4 + m
                            for half in range(2):
                                unit = m * 2 + half
                                cls = hidden_plan[lw * 16 + e * 2 + half]
                                p_z = z_ps.tile([128, CHUNK], F32, tag="z",
                                                name=f"z{c}{quad}{l}{m}{half}",
                                                bufs=z_bufs)
                                for si in range(NSUB):
                                    for kc in range(2):
                                        wc = ((e * 2 + kc) * 2 + half) * 128
                                        ru = m * 2 + kc
                                        nc.tensor.matmul(
                                            p_z[:, si * SUB:(si + 1) * SUB],
                                            t_wh[lw][:, wc:wc + 128],
                                            t_h[rpar][:, ru * UW + si * SUB:
                                                       ru * UW + (si + 1) * SUB],
                                            start=(kc == 0), stop=(kc == 1))
                                chc = C_BH + lw * 16 + e * 2 + half
                                chr_ = C_BHR + lw * 16 + e * 2 + half
                                hsl = t_h[wpar][:, unit * UW:(unit + 1) * UW]
                                if cls in (WRAP_ACT, WRAP_DVE):
                                    t_v = v_pool.tile(
                                        [128, CHUNK], F32, tag="vh", bufs=vh_bufs,
                                        name=f"vh{c}{quad}{l}{m}{half}")
                                    nc.vector._custom_dve(
                                        AMW, out=t_v[:], in0=p_z[:],
                                        s0=1.0,
                                        s1=t_cst[:, chc:chc + 1],
                                        imm2=MAGIC)
                                    if cls == WRAP_ACT:
                                        nc.scalar.activation(
                                            hsl, t_v[:], AT.Sin,
                                            bias=0.0, scale=TWO_PI)
                                    else:
                                        nc.vector._custom_dve(
                                            SIN7, out=hsl, in0=t_v[:],
                                            in1=ap_vstd,
                                            s0=0.0, s1=R_STD, imm2=U_STD)
                                elif cls == SKIP_ACT:
                                    nc.scalar.activation(
                                        hsl, p_z[:], AT.Sin,
                                        bias=t_cst[:, chr_:chr_ + 1],
                                        scale=TWO_PI)
                                else:  # SKIP_DVE
                                    nc.vector._custom_dve(
                                        SIN7, out=hsl, in0=p_z[:],
                                        in1=ap_vwide,
                                        s0=t_cst[:, chc:chc + 1],
                                        s1=R_WIDE, imm2=U_WIDE)

                    # ---- output layer: long accumulation per subtile
                    for s in range(NSUB):
                        p_y = chunk_y[s]
                        for m in range(4):
                            e = quad * 4 + m
                            for kc in range(2):
                                ru = m * 2 + kc
                                blk = (e * 2 + kc) * 8
                                nc.tensor.matmul(
                                    p_y[:, :],
                                    t_wo8[:, blk:blk + 8],
                                    t_h[1][:, ru * UW + s * SUB:
                                           ru * UW + (s + 1) * SUB],
                                    start=(quad == 0 and m == 0 and kc == 0),
                                    stop=(quad == 1 and m == 3 and kc == 1),
                                    skip_group_check=True)

                # ---- combine
                t_w8 = io_pool.tile([8, CHUNK], F32, tag="w8")
                for s in range(NSUB):
                    nc.vector.scalar_tensor_tensor(
                        t_w8[:, s * SUB:(s + 1) * SUB], chunk_y[s][:], ap_bo,
                        t_u[:, (c * NSUB + s) * SUB:(c * NSUB + s + 1) * SUB],
                        ALU.add, ALU.mult)
                nco = CHUNK // 128
                p_num = z_ps.tile([128, CHUNK], F32, tag="z", name=f"pnum{c}",
                                  bufs=z_bufs)
                for col in range(nco):
                    nc.tensor.matmul(p_num[:, col:col + 1],
                                     t_w8[:, col * 128:(col + 1) * 128],
                                     ap_ones8, start=True, stop=True)
                nc.vector.tensor_tensor(
                    t_rso[:, OUTCOLS + c * nco:OUTCOLS + (c + 1) * nco],
                    p_num[:, 0:nco],
                    t_rso[:, c * nco:(c + 1) * nco], ALU.mult)

            nc.sync.dma_start(d_out[:, :], t_rso[:, OUTCOLS:2 * OUTCOLS])

    nc.compile()
    return nc


def _plan_from_bounds(hb):
    """hb: [3][E][2] max |z+b| in turns (host-measured, pre-slack).
    Returns (hidden_plan, l0_plan) int tuples balancing per-chunk engine
    busy (ns units from the TRN2 cost model)."""
    skip_ok = [hb[l][e][h] * RANGE_SLACK <= SKIP_BOUND
               for l in range(NHID) for e in range(E) for h in range(2)]
    n_skip = sum(skip_ok)
    n_wrap = 48 - n_skip

    A_PS, A_SB, S_SB, S_PS, ACT_T, POOL3 = 1192, 1127, 1127, 1192, 1038, 5161
    FIX_DVE, FIX_ACT = 1400, 2400

    import os
    p_force = os.environ.get("PLAN_P")
    best = None
    MAX_SD = 4       # wide-fit (~1e-3) tiles cap
    # Pool's 3-op layer-0 chains are long (4.5us/tile); the schedule runs
    # best with Pool lightly loaded (measured), so cap p.
    for p in ([int(p_force)] if p_force else range(6)):
        for s0 in range(17):
            for sw in range(n_wrap + 1):
                for sd in range(min(MAX_SD, n_skip) + 1):
                    dve = (n_wrap * A_PS + (16 - p) * A_SB
                           + (sw + s0) * S_SB + sd * S_PS + FIX_DVE)
                    act = (64 - sw - s0 - sd) * ACT_T + FIX_ACT
                    pool = p * POOL3
                    t = max(dve, act, pool)
                    err = sd * 4 + (sw + s0)  # error-stack tiebreak
                    if best is None or (t, err) < best[0]:
                        best = ((t, err), (p, s0, sw, sd))
    p, s0, sw, sd = best[1]

    # layer-0: LAST p columns on Pool (first experts stay on the fast DVE
    # path so the first hidden matmuls start early); sin7 on the last s0
    l0 = []
    for col in range(16):
        on_pool = col >= 16 - p
        use_sin7 = col < s0
        if on_pool:
            l0.append(L0_POOL_DVE if use_sin7 else L0_POOL_ACT)
        else:
            l0.append(L0_DVE_DVE if use_sin7 else L0_DVE_ACT)

    # hidden: skip tiles sorted by bound; sd smallest-bound ones on DVE-wide
    idxs = list(range(48))
    skip_idx = sorted((i for i in idxs if skip_ok[i]),
                      key=lambda i: hb[i // 16][(i % 16) // 2][i % 2])
    sd_set = set(skip_idx[:sd])
    wrap_idx = [i for i in idxs if not skip_ok[i]]
    # spread sin7 over the wrapped tiles evenly
    sw_set = set(wrap_idx[int(round(j * len(wrap_idx) / max(sw, 1)))]
                 for j in range(sw)) if sw else set()
    hidden = []
    for i in idxs:
        if skip_ok[i]:
            hidden.append(SKIP_DVE if i in sd_set else SKIP_ACT)
        else:
            hidden.append(WRAP_DVE if i in sw_set else WRAP_ACT)
    return tuple(hidden), tuple(l0)


LAST_RESULT = None  # BassKernelResults of the most recent run (for test.py)
LAST_PLAN = None


def kernel(x, gate_w, gate_b, w0, b0, wh, bh, wo, bo):
    x = np.asarray(x, dtype=np.float32)
    gate_w = np.asarray(gate_w, dtype=np.float32)
    gate_b = np.asarray(gate_b, dtype=np.float32)
    w0 = np.asarray(w0, dtype=np.float32)
    b0 = np.asarray(b0, dtype=np.float32)
    wh = np.asarray(wh, dtype=np.float32)
    bh = np.asarray(bh, dtype=np.float32)
    wo = np.asarray(wo, dtype=np.float32)
    bo = np.asarray(bo, dtype=np.float32)

    # host forward pass: per-(layer, expert, half) |z + b| bound (turns)
    hb = np.zeros((NHID, E, 2), dtype=np.float64)
    xf0 = x.reshape(-1)
    for lo in range(0, xf0.size, 8192):
        xs = xf0[lo:lo + 8192]
        h = np.sin(OMEGA0 * (w0[:, :, 0:1] * xs[None, None, :]
                             + b0[:, :, None])).astype(np.float32)
        for l in range(NHID):
            z = SC * (np.einsum('egh,eht->egt', wh[l], h, dtype=np.float32)
                      + bh[l][:, :, None]).astype(np.float32)
            for e in range(E):
                for hf in range(2):
                    r = float(np.abs(z[e, hf * 128:(hf + 1) * 128]).max())
                    hb[l][e][hf] = max(hb[l][e][hf], r)
            h = np.sin(TWO_PI * z).astype(np.float32)

    plan = _plan_from_bounds(hb)
    global LAST_PLAN
    LAST_PLAN = plan
    hidden_plan, l0_plan = plan

    # per-source-tile poly gain (h tiles computed via SIN7 hold sin/K)
    gain = np.ones((NLAYERS, E, 2), dtype=np.float64)
    for col in range(16):
        if l0_plan[col] in (L0_POOL_DVE, L0_DVE_DVE):
            gain[0][col // 2][col % 2] = K_STD
    for i in range(48):
        cls = hidden_plan[i]
        l, e, hf = i // 16, (i % 16) // 2, i % 2
        if cls == WRAP_DVE:
            gain[l + 1][e][hf] = K_STD
        elif cls == SKIP_DVE:
            gain[l + 1][e][hf] = K_WIDE

    # ---- host packing
    whp = np.zeros((128, NHID * 4096), dtype=np.float16)
    for l in range(NHID):
        for e in range(E):
            for kc in range(2):
                for mc in range(2):
                    colbase = l * 4096 + ((e * 2 + kc) * 2 + mc) * 128
                    blk = (SC * wh[l, e, mc * 128:(mc + 1) * 128,
                                   kc * 128:(kc + 1) * 128]).T  # [k, m]
                    whp[:, colbase:colbase + 128] = (
                        blk * gain[l][e][kc]).astype(np.float16)

    wo8p = np.zeros((128, 128), dtype=np.float16)
    for e in range(E):
        for kc in range(2):
            wo8p[:, (e * 2 + kc) * 8 + e] = (
                wo[e, 0, kc * 128:(kc + 1) * 128] * gain[3][e][kc]
            ).astype(np.float16)

    consts = np.zeros((128, CONSTW), dtype=np.float32)
    for e in range(E):
        for half in range(2):
            col = e * 2 + half
            consts[:, C_A0 + col] = SC * w0[e, half * 128:(half + 1) * 128, 0]
            consts[:, C_C0 + col] = SC * b0[e, half * 128:(half + 1) * 128]
    for l in range(NHID):
        for e in range(E):
            for half in range(2):
                bcol = SC * bh[l, e, half * 128:(half + 1) * 128]
                consts[:, C_BH + l * 16 + e * 2 + half] = bcol
                consts[:, C_BHR + l * 16 + e * 2 + half] = TWO_PI * bcol
    consts[0:8, C_GB] = gate_b
    consts[0:8, C_BO] = bo[:, 0]
    consts[0:8, C_ONES8] = 1.0
    consts[:, C_VSTD] = V_STD
    consts[:, C_VWIDE] = V_WIDE
    consts[0, C_GW:C_GW + 8] = gate_w[:, 0]
    consts[0, C_ONES1:C_ONES1 + 128] = 1.0

    g16 = np.zeros((1, 16), dtype=np.float16)
    g16[0, 0:8] = gate_w[:, 0].astype(np.float16)

    xf = x.reshape(-1)                      # [B*N]
    in_maps = []
    for j in range(NCORES):
        xc = xf[j * PTS:(j + 1) * PTS].reshape(1, PTS).copy()
        in_maps.append({"x": xc, "x16": xc.astype(np.float16), "g16": g16,
                        "wh": whp, "wo8": wo8p, "consts": consts})

    if plan not in _BUILD_CACHE:
        _BUILD_CACHE[plan] = _build(plan)
    nc = _BUILD_CACHE[plan]

    global LAST_RESULT
    LAST_RESULT = run_bass_kernel_spmd(nc, in_maps, list(range(NCORES)))
    res = LAST_RESULT.results
    parts = []
    for j in range(NCORES):
        outT = res[j]["out"]                # [128, 32]
        parts.append(outT.T.reshape(-1))    # point t = col*128 + p
    full = np.concatenate(parts).astype(np.float32)
    return full.reshape(B, N, 1)


# revision 67
# speedup vs baseline: 1.5085x; 1.0906x over previous
"""MoE-SIREN (nn_MoE_36146444763329) Trainium2 Bass kernel, v2.

Dense MoE: 8 SIREN experts (1->256->256->256->256->1, sin(30*) activations),
softmax gate over experts, weighted combine. B=2, N=16384 points.

Data-parallel over 8 NeuronCores (4096 points each), no collectives.

v2 engine plan (per chunk of 1024 points; tile = [128, 1024]):
  - PE: all matmuls (hidden K=256 via 2 fp32r chunks, output M=8 padded,
    gate, x broadcast).  Angle math in "turns" (weights pre-scaled by
    omega0/2pi on host) so wrap = round-to-nearest subtraction.
  - DVE: AFFINE_MAGIC_WRAP custom op (1 pass: ym = in*s0 + s1, out =
    ym - rne(ym) via the 1.5*2^23 magic add) for both layer-0 (from SBUF
    x-broadcast) and hidden pre-activations (from PSUM); SIN7_PROD custom
    op (deg-7 odd minimax sine in factored form, poly gain folded into the
    next layer's weights) takes a slice of the sin work.
  - ACT: bulk of the sins; tiles whose |pre-activation| stays within the
    hardware Sin table's accurate range (~3.5 rad) skip the wrap and go
    PSUM -> sin directly with a per-partition bias.
  - Pool: layer-0 wrap via 3 standard ops for most (e, half) columns.
A tiny host-side balancer picks per-tile engine assignment from these
measured per-tile ranges; the result is cached per plan.
"""
import numpy as np

import concourse.bass as bass
import concourse.mybir as mybir
import concourse.tile as tile
from concourse import bacc
from concourse.bass_utils import run_bass_kernel_spmd

import concourse.dve_ops as dve_ops
from concourse.dve_ops import DveOp
from concourse.dve_spec import (Spec, Src0, C0, C1, C2, C3, lower,
                                _spill_c3_to_src1)
from concourse.dve_uop import DveOpSpec

F32 = mybir.dt.float32
F32R = mybir.dt.float32r
F16 = mybir.dt.float16
AT = mybir.ActivationFunctionType
ALU = mybir.AluOpType

B, N, E, H, NLAYERS = 2, 16384, 8, 256, 4
OMEGA0 = 30.0
NCORES = 8
PTS = B * N // NCORES            # 4096 points per core
CHUNK = 1024
NCHUNK = PTS // CHUNK            # 4
SUB = 512                        # matmul moving free dim
NSUB = CHUNK // SUB              # 2
NHID = NLAYERS - 1               # 3 hidden layers
TWO_PI = float(2.0 * np.pi)
SC = float(OMEGA0 / (2.0 * np.pi))   # pre-scale: radians -> turns
MAGIC = float(np.float32(1.5 * 2 ** 23))
OUTCOLS = PTS // 128             # 32 columns of transposed output per core
UW = CHUNK                       # one (m, half) slab width
NB = 8 * UW                      # big-tile width: 8 units

# deg-7 odd minimax sine fits, factored p(y) = K*y*(t-R)*(t^2 + U t + V),
# t = y^2.  std: |y| <= 0.5 (post-wrap), err 2.5e-4.  wide: |y| <= 0.585
# (wrap-skipped tiles incl. slack), err 9.4e-4.  K folds into the consumer
# weights host-side.
K_STD = -56.08679675286569
R_STD = 0.24996040959546928
U_STD = -1.1395020867858219
V_STD = 0.44785087867283263
K_WIDE = -49.32072447535861
R_WIDE = 0.24994924240031763
U_WIDE = -1.2688422239794543
V_WIDE = 0.5083875825756747

SKIP_BOUND = 0.62                # turns (~3.90 rad); tile range (w/ slack)
                                 # below this may use the ACT table without a
                                 # wrap (hw table tail err <= ~4e-3, and only
                                 # at a tile's few extreme-|z| points)
SD_BOUND = 0.59                  # wide SIN7 poly fit domain (turns)
RANGE_SLACK = 1.02               # host-measured range -> hw guard factor

# consts tile column layout ([128, 512] fp32)
C_A0 = 0       # 16 cols: layer0 scale SC*w0, col = e*2+half
C_C0 = 16      # 16 cols: layer0 bias  SC*b0
C_BH = 32      # 48 cols: hidden bias (turns)   (l-1)*16 + e*2 + half
C_BHR = 80     # 48 cols: hidden bias (radians, *2pi)
C_GB = 128     # 1 col: gate bias (partitions 0..7)
C_BO = 129     # 1 col: output bias (partitions 0..7)
C_ONES8 = 130  # 1 col: ones (partitions 0..7)
C_VSTD = 131   # 1 col: V_STD on all partitions
C_VWIDE = 132  # 1 col: V_WIDE on all partitions
C_GW = 136     # row 0, cols 136..143: gate weights (lhsT [1,8])
C_ONES1 = 144  # row 0, cols 144..271: ones (lhsT [1,128])
C_WO8 = 272    # 128 cols: zero-padded output lhsT blocks [(e*2+kc)*8 + e']
CONSTW = 512

# hidden tile classes
WRAP_ACT, WRAP_DVE, SKIP_ACT, SKIP_DVE = 0, 1, 2, 3
# layer-0 tile classes: (wrap engine, sin engine)
L0_POOL_ACT, L0_DVE_ACT, L0_POOL_DVE, L0_DVE_DVE = 0, 1, 2, 3

_BUILD_CACHE: dict = {}


def _register(name, spec, rd1):
    """Idempotently add a DveOp to the module-level registry; sha computed
    at runtime so the pin always matches this environment's lower()."""
    for o in dve_ops.OPS:
        if o.name == name:
            return o
    row = dve_ops._CUSTOM_DVE_ROW_BASE + len(dve_ops.OPS)
    assert row < 0x20
    dve_ops._SUB_OPCODE_FOR_NAME[name] = row
    shas = {}
    for ver in ("v3", "v4"):
        s = DveOpSpec(name=name, opcode=row, uops=lower(spec, ver=ver),
                      rd1_en=rd1)
        shas[ver] = s.sha(ver)
    op = DveOp(name, spec, subdim=False, uops_sha=shas)
    dve_ops.OPS.append(op)
    dve_ops.CUSTOM_DVE_SPECS[name] = spec
    return op


def _make_ops():
    _ym = Src0 * C0 + C1
    _amw = _ym - ((_ym + C2) - C2)

    def _amw_ref(in0, in1, s0, s1, imm2):
        ym = (in0.astype(np.float32) * np.float32(s0)).astype(np.float32)
        ym = (ym + np.float32(s1)).astype(np.float32)
        k = ((ym + np.float32(imm2)).astype(np.float32)
             - np.float32(imm2)).astype(np.float32)
        return ym - k

    amw = _register("AFFINE_MAGIC_WRAP_ANT", Spec(body=_amw, reference=_amw_ref),
                    False)

    _y2 = Src0 + C0
    _t = _y2 * _y2
    _sin = _spill_c3_to_src1(((_t - C1) * ((_t + C2) * _t + C3)) * _y2)

    def _sin_ref(in0, in1, s0, s1, imm2):
        ym = in0.astype(np.float32) + np.float32(s0)
        t = ym * ym
        v = np.asarray(in1, np.float32).reshape(-1, 1)
        return ((t - np.float32(s1)) * ((t + np.float32(imm2)) * t + v)) * ym

    sin7 = _register("SIN7_PROD_ANT", Spec(body=_sin, reference=_sin_ref), True)
    return amw, sin7


AMW, SIN7 = _make_ops()


def _build(plan, z_bufs=3, y_bufs=2, hl0_bufs=2, h0_bufs=2, h1_bufs=2,
           vh_bufs=4):
    """hl0_bufs=0 merges the layer-0 slab into the par-0 slab."""
    hidden_plan, l0_plan = plan
    nc = bacc.Bacc("TRN2", target_bir_lowering=False, debug=False,
                   num_devices=NCORES)

    d_x = nc.dram_tensor("x", [1, PTS], F32, kind="ExternalInput")
    d_x16 = nc.dram_tensor("x16", [1, PTS], F16, kind="ExternalInput")
    d_g16 = nc.dram_tensor("g16", [1, 16], F16, kind="ExternalInput")
    d_wh = nc.dram_tensor("wh", [128, NHID * 4096], F16, kind="ExternalInput")
    d_wo8 = nc.dram_tensor("wo8", [128, 128], F16, kind="ExternalInput")
    d_consts = nc.dram_tensor("consts", [128, CONSTW], F32,
                              kind="ExternalInput")
    d_out = nc.dram_tensor("out", [128, OUTCOLS], F32, kind="ExternalOutput")

    with tile.TileContext(nc) as tc:
        with (
            tc.tile_pool(name="cst", bufs=1) as cst_pool,
            tc.tile_pool(name="whp", bufs=1) as wh_pool,
            tc.tile_pool(name="io", bufs=1) as io_pool,
            tc.tile_pool(name="hbuf", bufs=1) as h_pool,
            tc.tile_pool(name="vbuf", bufs=1) as v_pool,
            tc.tile_pool(name="tmp", bufs=1) as tmp_pool,
            tc.tile_pool(name="zps", bufs=1, space="PSUM") as z_ps,
            tc.tile_pool(name="yps", bufs=1, space="PSUM") as y_ps,
        ):
            t_cst = cst_pool.tile([128, CONSTW], F32, tag="consts")
            nc.sync.dma_start(t_cst[:], d_consts[:, :])
            t_x16 = io_pool.tile([1, PTS], F16, tag="x16")
            nc.sync.dma_start(t_x16[:], d_x16[0:1, :])
            t_g16 = io_pool.tile([1, 16], F16, tag="g16")
            nc.sync.dma_start(t_g16[:], d_g16[0:1, :])
            ap_gb = t_cst[0:8, C_GB:C_GB + 1]
            ap_bo = t_cst[0:8, C_BO:C_BO + 1]
            ap_ones8 = t_cst[0:8, C_ONES8:C_ONES8 + 1]
            ap_gw = t_g16[0:1, 0:8]
            ap_ones1 = t_cst[0:1, C_ONES1:C_ONES1 + 128]
            ap_vstd = t_cst[:, C_VSTD:C_VSTD + 1]
            ap_vwide = t_cst[:, C_VWIDE:C_VWIDE + 1]

            # hidden + output weights in fp16 (same PE rate, half the SBUF)
            t_wh = []
            for l in range(NHID):
                w = wh_pool.tile([128, 4096], F16, tag=f"wh{l}", name=f"wh{l}")
                nc.gpsimd.dma_start(w[:], d_wh[:, l * 4096:(l + 1) * 4096])
                t_wh.append(w)
            t_wo8 = wh_pool.tile([128, 128], F16, tag="wo8", name="wo8")
            nc.gpsimd.dma_start(t_wo8[:], d_wo8[:, :])

            t_rso = tmp_pool.tile([128, 2 * OUTCOLS], F32, tag="rso")
            t_u = io_pool.tile([8, PTS], F32, tag="u")
            t_xc = {}

            def emit_gate():
                # whole-run gate, emitted inside chunk 0 so it doesn't sit at
                # the engines' queue heads before any layer-0 work
                for s in range(PTS // SUB):
                    p_zg = y_ps.tile([8, SUB], F32, tag="y", name=f"zg{s}",
                                     bufs=y_bufs)
                    nc.tensor.matmul(p_zg[:], ap_gw,
                                     t_x16[:, s * SUB:(s + 1) * SUB],
                                     start=True, stop=True)
                    nc.scalar.activation(t_u[:, s * SUB:(s + 1) * SUB],
                                         p_zg[:], AT.Exp, bias=ap_gb,
                                         scale=1.0)
                p_den = z_ps.tile([128, CHUNK], F32, tag="z", name="pden",
                                  bufs=z_bufs)
                for col in range(OUTCOLS):
                    nc.tensor.matmul(p_den[:, col:col + 1],
                                     t_u[:, col * 128:(col + 1) * 128],
                                     ap_ones8, start=True, stop=True)
                nc.vector.reciprocal(t_rso[:, 0:OUTCOLS], p_den[:, 0:OUTCOLS])

            # x broadcast tiles, pipelined one chunk ahead (PSUM -> ACT copy)
            t_xb = {}

            def emit_xb(c):
                t_xc[c] = io_pool.tile([1, CHUNK], F32, tag="xc", bufs=2,
                                       name=f"xc{c}")
                nc.sync.dma_start(t_xc[c][:],
                                  d_x[0:1, c * CHUNK:(c + 1) * CHUNK])
                t = io_pool.tile([128, CHUNK], F32, tag="xb", bufs=2,
                                 name=f"xb{c}")
                nc.gpsimd.partition_broadcast(t[:], t_xc[c][:], channels=128)
                t_xb[c] = t

            emit_xb(0)

            # layer-0 software pipelining: the WRAP ops (Pool 3-op chains and
            # DVE AMW/SIN7, whose inputs are always ready) are hoisted one
            # quad ahead in their engine queues; the ACT sins stay at the
            # quad's own position (hoisting them risks head-of-line blocking
            # on the producing engine).
            slabs = {}

            def emit_l0_wrap(c, quad):
                t_h = {0: h_pool.tile([128, NB], F16, tag="hb0",
                                      bufs=h0_bufs,
                                      name=f"hb{c}_{quad}_0"),
                       1: h_pool.tile([128, NB], F16, tag="hb1",
                                      bufs=h1_bufs,
                                      name=f"hb{c}_{quad}_1")}
                t_h["l0"] = (h_pool.tile([128, NB], F16, tag="hl0",
                                         bufs=hl0_bufs,
                                         name=f"hl{c}_{quad}")
                             if hl0_bufs else t_h[0])
                slabs[(c, quad)] = t_h
                for m in range(4):
                    e = quad * 4 + m
                    for half in range(2):
                        col = e * 2 + half
                        unit = m * 2 + half
                        cls = l0_plan[col]
                        ap_a = t_cst[:, C_A0 + col:C_A0 + col + 1]
                        ap_c = t_cst[:, C_C0 + col:C_C0 + col + 1]
                        hsl = t_h["l0"][:, unit * UW:(unit + 1) * UW]
                        # wrap written straight into the fp16 slab slot, then
                        # sin in-place: the slab double-buffering is the
                        # producer runahead
                        if cls in (L0_POOL_ACT, L0_POOL_DVE):
                            t_zb = tmp_pool.tile([128, UW], F32, tag="zb",
                                                 bufs=2,
                                                 name=f"zb{c}{quad}{col}")
                            nc.gpsimd.tensor_scalar(
                                t_zb[:], t_xb[c][:], ap_a, ap_c,
                                ALU.mult, ALU.add)
                            t_k = tmp_pool.tile([128, UW], F32, tag="k",
                                                bufs=2,
                                                name=f"k{c}{quad}{col}")
                            nc.gpsimd.tensor_scalar(
                                t_k[:], t_zb[:], MAGIC, MAGIC,
                                ALU.add, ALU.subtract)
                            nc.gpsimd.tensor_tensor(
                                hsl, t_zb[:], t_k[:], ALU.subtract)
                        else:
                            nc.vector._custom_dve(
                                AMW, out=hsl, in0=t_xb[c][:],
                                s0=ap_a, s1=ap_c, imm2=MAGIC)
                        if cls in (L0_POOL_DVE, L0_DVE_DVE):
                            nc.vector._custom_dve(
                                SIN7, out=hsl, in0=hsl, in1=ap_vstd,
                                s0=0.0, s1=R_STD, imm2=U_STD)

            def emit_l0_sin(c, quad):
                t_h = slabs[(c, quad)]
                for m in range(4):
                    e = quad * 4 + m
                    for half in range(2):
                        col = e * 2 + half
                        unit = m * 2 + half
                        cls = l0_plan[col]
                        if cls in (L0_POOL_ACT, L0_DVE_ACT):
                            hsl = t_h["l0"][:, unit * UW:(unit + 1) * UW]
                            nc.scalar.activation(hsl, hsl, AT.Sin,
                                                 bias=0.0, scale=TWO_PI)

            for c in range(NCHUNK):
                if c + 1 < NCHUNK:
                    emit_xb(c + 1)
                if c == 0:
                    emit_gate()

                chunk_y = [y_ps.tile([8, SUB], F32, tag="y", name=f"y{c}_{s}",
                                     bufs=y_bufs)
                           for s in range(NSUB)]

                for quad in range(2):
                    emit_l0_wrap(c, quad)
                    emit_l0_sin(c, quad)
                    t_h = slabs.pop((c, quad))

                    # ---- hidden layers, 4-expert staggered
                    for l in range(1, NLAYERS):
                        lw = l - 1
                        rpar = "l0" if l == 1 else (l - 1) & 1
                        wpar = l & 1
                        for m in range(4):
                            e = quad * 4 + m
                            for half in range(2):
                                unit = m * 2 + half
                                cls = hidden_plan[lw * 16 + e * 2 + half]
                                p_z = z_ps.tile([128, CHUNK], F32, tag="z",
                                                name=f"z{c}{quad}{l}{m}{half}",
                                                bufs=z_bufs)
                                for si in range(NSUB):
                                    for kc in range(2):
                                        wc = ((e * 2 + kc) * 2 + half) * 128
                                        ru = m * 2 + kc
                                        nc.tensor.matmul(
                                            p_z[:, si * SUB:(si + 1) * SUB],
                                            t_wh[lw][:, wc:wc + 128],
                                            t_h[rpar][:, ru * UW + si * SUB:
                                                       ru * UW + (si + 1) * SUB],
                                            start=(kc == 0), stop=(kc == 1))
                                chc = C_BH + lw * 16 + e * 2 + half
                                chr_ = C_BHR + lw * 16 + e * 2 + half
                                hsl = t_h[wpar][:, unit * UW:(unit + 1) * UW]
                                if cls in (WRAP_ACT, WRAP_DVE):
                                    t_v = v_pool.tile(
                                        [128, CHUNK], F32, tag="vh", bufs=vh_bufs,
                                        name=f"vh{c}{quad}{l}{m}{half}")
                                    nc.vector._custom_dve(
                                        AMW, out=t_v[:], in0=p_z[:],
                                        s0=1.0,
                                        s1=t_cst[:, chc:chc + 1],
                                        imm2=MAGIC)
                                    if cls == WRAP_ACT:
                                        nc.scalar.activation(
                                            hsl, t_v[:], AT.Sin,
                                            bias=0.0, scale=TWO_PI)
                                    else:
                                        nc.vector._custom_dve(
                                            SIN7, out=hsl, in0=t_v[:],
                                            in1=ap_vstd,
                                            s0=0.0, s1=R_STD, imm2=U_STD)
                                elif cls == SKIP_ACT:
                                    nc.scalar.activation(
                                        hsl, p_z[:], AT.Sin,
                                        bias=t_cst[:, chr_:chr_ + 1],
                                        scale=TWO_PI)
                                else:  # SKIP_DVE
                                    nc.vector._custom_dve(
                                        SIN7, out=hsl, in0=p_z[:],
                                        in1=ap_vwide,
                                        s0=t_cst[:, chc:chc + 1],
                                        s1=R_WIDE, imm2=U_WIDE)

                    # ---- output layer: long accumulation per subtile
                    for s in range(NSUB):
                        p_y = chunk_y[s]
                        for m in range(4):
                            e = quad * 4 + m
                            for kc in range(2):
                                ru = m * 2 + kc
                                blk = (e * 2 + kc) * 8
                                nc.tensor.matmul(
                                    p_y[:, :],
                                    t_wo8[:, blk:blk + 8],
                                    t_h[1][:, ru * UW + s * SUB:
                                           ru * UW + (s + 1) * SUB],
                                    start=(quad == 0 and m == 0 and kc == 0),
                                    stop=(quad == 1 and m == 3 and kc == 1),
                                    skip_group_check=True)

                # ---- combine
                t_w8 = io_pool.tile([8, CHUNK], F32, tag="w8")
                for s in range(NSUB):
                    nc.vector.scalar_tensor_tensor(
                        t_w8[:, s * SUB:(s + 1) * SUB], chunk_y[s][:], ap_bo,
                        t_u[:, (c * NSUB + s) * SUB:(c * NSUB + s + 1) * SUB],
                        ALU.add, ALU.mult)
                nco = CHUNK // 128
                p_num = z_ps.tile([128, CHUNK], F32, tag="z", name=f"pnum{c}",
                                  bufs=z_bufs)
                for col in range(nco):
                    nc.tensor.matmul(p_num[:, col:col + 1],
                                     t_w8[:, col * 128:(col + 1) * 128],
                                     ap_ones8, start=True, stop=True)
                nc.vector.tensor_tensor(
                    t_rso[:, OUTCOLS + c * nco:OUTCOLS + (c + 1) * nco],
                    p_num[:, 0:nco],
                    t_rso[:, c * nco:(c + 1) * nco], ALU.mult)

            nc.sync.dma_start(d_out[:, :], t_rso[:, OUTCOLS:2 * OUTCOLS])

    nc.compile()
    return nc


def _plan_from_bounds(hb):
    """hb: [3][E][2] max |z+b| in turns (host-measured, pre-slack).
    Returns (hidden_plan, l0_plan) int tuples balancing per-chunk engine
    busy (ns units from the TRN2 cost model)."""
    flat = [hb[l][e][h] * RANGE_SLACK
            for l in range(NHID) for e in range(E) for h in range(2)]
    skip_ok = [b <= SKIP_BOUND for b in flat]
    sd_ok = [b <= SD_BOUND for b in flat]
    n_skip = sum(skip_ok)
    n_wrap = 48 - n_skip

    A_PS, A_SB, S_SB, S_PS, ACT_T, POOL3 = 1192, 1127, 1127, 1192, 1038, 5161
    FIX_DVE, FIX_ACT = 1400, 2400

    import os
    p_force = os.environ.get("PLAN_P")
    sw_force = os.environ.get("PLAN_SW")
    best = None
    MAX_SD = 4       # wide-fit (~1e-3) tiles cap
    # Pool handles only the x partition-broadcasts (keeps it on one GPSIMD
    # library, no reloads); measured best with its 3-op layer-0 path unused.
    for p in ([int(p_force)] if p_force else [0]):
        for s0 in range(17):
            for sw in ([int(sw_force)] if sw_force else range(n_wrap + 1)):
                n_sd = sum(1 for i in range(48) if skip_ok[i] and sd_ok[i])
                for sd in range(min(MAX_SD, n_sd) + 1):
                    dve = (n_wrap * A_PS + (16 - p) * A_SB
                           + (sw + s0) * S_SB + sd * S_PS + FIX_DVE)
                    act = (64 - sw - s0 - sd) * ACT_T + FIX_ACT
                    pool = p * POOL3
                    t = max(dve, act, pool)
                    err = sd * 4 + (sw + s0)  # error-stack tiebreak
                    if best is None or (t, err) < best[0]:
                        best = ((t, err), (p, s0, sw, sd))
    p, s0, sw, sd = best[1]

    # layer-0: LAST p columns on Pool (first experts stay on the fast DVE
    # path so the first hidden matmuls start early); sin7 on the last s0
    l0 = []
    for col in range(16):
        on_pool = col >= 16 - p
        use_sin7 = col < s0
        if on_pool:
            l0.append(L0_POOL_DVE if use_sin7 else L0_POOL_ACT)
        else:
            l0.append(L0_DVE_DVE if use_sin7 else L0_DVE_ACT)

    # hidden: skip tiles sorted by bound; sd smallest-bound ones (within the
    # wide poly's fit domain) on DVE-wide
    idxs = list(range(48))
    skip_idx = sorted((i for i in idxs if skip_ok[i] and sd_ok[i]),
                      key=lambda i: hb[i // 16][(i % 16) // 2][i % 2])
    sd_set = set(skip_idx[:sd])
    wrap_idx = [i for i in idxs if not skip_ok[i]]
    # spread sin7 over the wrapped tiles evenly
    sw_set = set(wrap_idx[int(round(j * len(wrap_idx) / max(sw, 1)))]
                 for j in range(sw)) if sw else set()
    hidden = []
    for i in idxs:
        if skip_ok[i]:
            hidden.append(SKIP_DVE if i in sd_set else SKIP_ACT)
        else:
            hidden.append(WRAP_DVE if i in sw_set else WRAP_ACT)
    return tuple(hidden), tuple(l0)


LAST_RESULT = None  # BassKernelResults of the most recent run (for test.py)
LAST_PLAN = None


def kernel(x, gate_w, gate_b, w0, b0, wh, bh, wo, bo):
    x = np.asarray(x, dtype=np.float32)
    gate_w = np.asarray(gate_w, dtype=np.float32)
    gate_b = np.asarray(gate_b, dtype=np.float32)
    w0 = np.asarray(w0, dtype=np.float32)
    b0 = np.asarray(b0, dtype=np.float32)
    wh = np.asarray(wh, dtype=np.float32)
    bh = np.asarray(bh, dtype=np.float32)
    wo = np.asarray(wo, dtype=np.float32)
    bo = np.asarray(bo, dtype=np.float32)

    # host forward pass: per-(layer, expert, half) |z + b| bound (turns)
    hb = np.zeros((NHID, E, 2), dtype=np.float64)
    xf0 = x.reshape(-1)
    for lo in range(0, xf0.size, 8192):
        xs = xf0[lo:lo + 8192]
        h = np.sin(OMEGA0 * (w0[:, :, 0:1] * xs[None, None, :]
                             + b0[:, :, None])).astype(np.float32)
        for l in range(NHID):
            z = SC * (np.einsum('egh,eht->egt', wh[l], h, dtype=np.float32)
                      + bh[l][:, :, None]).astype(np.float32)
            for e in range(E):
                for hf in range(2):
                    r = float(np.abs(z[e, hf * 128:(hf + 1) * 128]).max())
                    hb[l][e][hf] = max(hb[l][e][hf], r)
            h = np.sin(TWO_PI * z).astype(np.float32)

    plan = _plan_from_bounds(hb)
    global LAST_PLAN
    LAST_PLAN = plan
    hidden_plan, l0_plan = plan

    # per-source-tile poly gain (h tiles computed via SIN7 hold sin/K)
    gain = np.ones((NLAYERS, E, 2), dtype=np.float64)
    for col in range(16):
        if l0_plan[col] in (L0_POOL_DVE, L0_DVE_DVE):
            gain[0][col // 2][col % 2] = K_STD
    for i in range(48):
        cls = hidden_plan[i]
        l, e, hf = i // 16, (i % 16) // 2, i % 2
        if cls == WRAP_DVE:
            gain[l + 1][e][hf] = K_STD
        elif cls == SKIP_DVE:
            gain[l + 1][e][hf] = K_WIDE

    # ---- host packing
    whp = np.zeros((128, NHID * 4096), dtype=np.float16)
    for l in range(NHID):
        for e in range(E):
            for kc in range(2):
                for mc in range(2):
                    colbase = l * 4096 + ((e * 2 + kc) * 2 + mc) * 128
                    blk = (SC * wh[l, e, mc * 128:(mc + 1) * 128,
                                   kc * 128:(kc + 1) * 128]).T  # [k, m]
                    whp[:, colbase:colbase + 128] = (
                        blk * gain[l][e][kc]).astype(np.float16)

    wo8p = np.zeros((128, 128), dtype=np.float16)
    for e in range(E):
        for kc in range(2):
            wo8p[:, (e * 2 + kc) * 8 + e] = (
                wo[e, 0, kc * 128:(kc + 1) * 128] * gain[3][e][kc]
            ).astype(np.float16)

    consts = np.zeros((128, CONSTW), dtype=np.float32)
    for e in range(E):
        for half in range(2):
            col = e * 2 + half
            consts[:, C_A0 + col] = SC * w0[e, half * 128:(half + 1) * 128, 0]
            consts[:, C_C0 + col] = SC * b0[e, half * 128:(half + 1) * 128]
    for l in range(NHID):
        for e in range(E):
            for half in range(2):
                bcol = SC * bh[l, e, half * 128:(half + 1) * 128]
                consts[:, C_BH + l * 16 + e * 2 + half] = bcol
                consts[:, C_BHR + l * 16 + e * 2 + half] = TWO_PI * bcol
    consts[0:8, C_GB] = gate_b
    consts[0:8, C_BO] = bo[:, 0]
    consts[0:8, C_ONES8] = 1.0
    consts[:, C_VSTD] = V_STD
    consts[:, C_VWIDE] = V_WIDE
    consts[0, C_GW:C_GW + 8] = gate_w[:, 0]
    consts[0, C_ONES1:C_ONES1 + 128] = 1.0

    g16 = np.zeros((1, 16), dtype=np.float16)
    g16[0, 0:8] = gate_w[:, 0].astype(np.float16)

    xf = x.reshape(-1)                      # [B*N]
    in_maps = []
    for j in range(NCORES):
        xc = xf[j * PTS:(j + 1) * PTS].reshape(1, PTS).copy()
        in_maps.append({"x": xc, "x16": xc.astype(np.float16), "g16": g16,
                        "wh": whp, "wo8": wo8p, "consts": consts})

    if plan not in _BUILD_CACHE:
        _BUILD_CACHE[plan] = _build(plan)
    nc = _BUILD_CACHE[plan]

    global LAST_RESULT
    LAST_RESULT = run_bass_kernel_spmd(nc, in_maps, list(range(NCORES)))
    res = LAST_RESULT.results
    parts = []
    for j in range(NCORES):
        outT = res[j]["out"]                # [128, 32]
        parts.append(outT.T.reshape(-1))    # point t = col*128 + p
    full = np.concatenate(parts).astype(np.float32)
    return full.reshape(B, N, 1)


# revision 92
# speedup vs baseline: 1.5448x; 1.0241x over previous
"""MoE-SIREN (nn_MoE_36146444763329) Trainium2 Bass kernel, v2.

Dense MoE: 8 SIREN experts (1->256->256->256->256->1, sin(30*) activations),
softmax gate over experts, weighted combine. B=2, N=16384 points.

Data-parallel over 8 NeuronCores (4096 points each), no collectives.

v2 engine plan (per chunk of 1024 points; tile = [128, 1024]):
  - PE: all matmuls (hidden K=256 via 2 fp32r chunks, output M=8 padded,
    gate, x broadcast).  Angle math in "turns" (weights pre-scaled by
    omega0/2pi on host) so wrap = round-to-nearest subtraction.
  - DVE: AFFINE_MAGIC_WRAP custom op (1 pass: ym = in*s0 + s1, out =
    ym - rne(ym) via the 1.5*2^23 magic add) for both layer-0 (from SBUF
    x-broadcast) and hidden pre-activations (from PSUM); SIN7_PROD custom
    op (deg-7 odd minimax sine in factored form, poly gain folded into the
    next layer's weights) takes a slice of the sin work.
  - ACT: bulk of the sins; tiles whose |pre-activation| stays within the
    hardware Sin table's accurate range (~3.5 rad) skip the wrap and go
    PSUM -> sin directly with a per-partition bias.
  - Pool: layer-0 wrap via 3 standard ops for most (e, half) columns.
A tiny host-side balancer picks per-tile engine assignment from these
measured per-tile ranges; the result is cached per plan.
"""
import numpy as np

import concourse.bass as bass
import concourse.mybir as mybir
import concourse.tile as tile
from concourse import bacc
from concourse.bass_utils import run_bass_kernel_spmd

import concourse.dve_ops as dve_ops
from concourse.dve_ops import DveOp
from concourse.dve_spec import (Spec, Src0, C0, C1, C2, C3, lower,
                                _spill_c3_to_src1)
from concourse.dve_uop import DveOpSpec

F32 = mybir.dt.float32
F32R = mybir.dt.float32r
F16 = mybir.dt.float16
AT = mybir.ActivationFunctionType
ALU = mybir.AluOpType

B, N, E, H, NLAYERS = 2, 16384, 8, 256, 4
OMEGA0 = 30.0
NCORES = 8
PTS = B * N // NCORES            # 4096 points per core
CHUNK = 1024
NCHUNK = PTS // CHUNK            # 4
SUB = 512                        # matmul moving free dim
NSUB = CHUNK // SUB              # 2
NHID = NLAYERS - 1               # 3 hidden layers
TWO_PI = float(2.0 * np.pi)
SC = float(OMEGA0 / (2.0 * np.pi))   # pre-scale: radians -> turns
MAGIC = float(np.float32(1.5 * 2 ** 23))
OUTCOLS = PTS // 128             # 32 columns of transposed output per core
UW = CHUNK                       # one (m, half) slab width
NB = 8 * UW                      # big-tile width: 8 units

# deg-7 odd minimax sine fits, factored p(y) = K*y*(t-R)*(t^2 + U t + V),
# t = y^2.  std: |y| <= 0.5 (post-wrap), err 2.5e-4.  wide: |y| <= 0.585
# (wrap-skipped tiles incl. slack), err 9.4e-4.  K folds into the consumer
# weights host-side.
K_STD = -56.08679675286569
R_STD = 0.24996040959546928
U_STD = -1.1395020867858219
V_STD = 0.44785087867283263
K_WIDE = -49.32072447535861
R_WIDE = 0.24994924240031763
U_WIDE = -1.2688422239794543
V_WIDE = 0.5083875825756747

SKIP_BOUND = 0.65                # turns (~4.08 rad); tile range (w/ slack)
                                 # below this may use the ACT table without a
                                 # wrap (hw table tail err <= ~7e-3, and only
                                 # at a tile's few extreme-|z| points; measured
                                 # end-to-end impact is in the noise)
SD_BOUND = 0.59                  # wide SIN7 poly fit domain (turns)
RANGE_SLACK = 1.02               # host-measured range -> hw guard factor

# consts tile column layout ([128, 512] fp32)
C_A0 = 0       # 16 cols: layer0 scale SC*w0, col = e*2+half
C_C0 = 16      # 16 cols: layer0 bias  SC*b0
C_BH = 32      # 48 cols: hidden bias (turns)   (l-1)*16 + e*2 + half
C_BHR = 80     # 48 cols: hidden bias (radians, *2pi)
C_GB = 128     # 1 col: gate bias (partitions 0..7)
C_BO = 129     # 1 col: output bias (partitions 0..7)
C_ONES8 = 130  # 1 col: ones (partitions 0..7)
C_VSTD = 131   # 1 col: V_STD on all partitions
C_VWIDE = 132  # 1 col: V_WIDE on all partitions
C_GW = 136     # row 0, cols 136..143: gate weights (lhsT [1,8])
C_ONES1 = 144  # row 0, cols 144..271: ones (lhsT [1,128])
C_WO8 = 272    # 128 cols: zero-padded output lhsT blocks [(e*2+kc)*8 + e']
CONSTW = 512

# hidden tile classes
WRAP_ACT, WRAP_DVE, SKIP_ACT, SKIP_DVE = 0, 1, 2, 3
# layer-0 tile classes: (wrap engine, sin engine)
L0_POOL_ACT, L0_DVE_ACT, L0_POOL_DVE, L0_DVE_DVE = 0, 1, 2, 3

_BUILD_CACHE: dict = {}


def _register(name, spec, rd1):
    """Idempotently add a DveOp to the module-level registry; sha computed
    at runtime so the pin always matches this environment's lower()."""
    for o in dve_ops.OPS:
        if o.name == name:
            return o
    row = dve_ops._CUSTOM_DVE_ROW_BASE + len(dve_ops.OPS)
    assert row < 0x20
    dve_ops._SUB_OPCODE_FOR_NAME[name] = row
    shas = {}
    for ver in ("v3", "v4"):
        s = DveOpSpec(name=name, opcode=row, uops=lower(spec, ver=ver),
                      rd1_en=rd1)
        shas[ver] = s.sha(ver)
    op = DveOp(name, spec, subdim=False, uops_sha=shas)
    dve_ops.OPS.append(op)
    dve_ops.CUSTOM_DVE_SPECS[name] = spec
    return op


def _make_ops():
    _ym = Src0 * C0 + C1
    _amw = _ym - ((_ym + C2) - C2)

    def _amw_ref(in0, in1, s0, s1, imm2):
        ym = (in0.astype(np.float32) * np.float32(s0)).astype(np.float32)
        ym = (ym + np.float32(s1)).astype(np.float32)
        k = ((ym + np.float32(imm2)).astype(np.float32)
             - np.float32(imm2)).astype(np.float32)
        return ym - k

    amw = _register("AFFINE_MAGIC_WRAP_ANT", Spec(body=_amw, reference=_amw_ref),
                    False)

    _y2 = Src0 + C0
    _t = _y2 * _y2
    _sin = _spill_c3_to_src1(((_t - C1) * ((_t + C2) * _t + C3)) * _y2)

    def _sin_ref(in0, in1, s0, s1, imm2):
        ym = in0.astype(np.float32) + np.float32(s0)
        t = ym * ym
        v = np.asarray(in1, np.float32).reshape(-1, 1)
        return ((t - np.float32(s1)) * ((t + np.float32(imm2)) * t + v)) * ym

    sin7 = _register("SIN7_PROD_ANT", Spec(body=_sin, reference=_sin_ref), True)
    return amw, sin7


AMW, SIN7 = _make_ops()


def _build(plan, z_bufs=3, y_bufs=2, hl0_bufs=2, h0_bufs=2, h1_bufs=2,
           vh_bufs=6):
    """hl0_bufs=0 merges the layer-0 slab into the par-0 slab.
    plan = (hidden_plan, l0_plan, tail_flips): tail_flips lists l=3 tile
    indices (32..47) that run SKIP_DVE in the LAST chunk only (their gain
    change is carried by the second wo8 copy)."""
    hidden_plan, l0_plan, tail_flips = plan
    tail_set = set(tail_flips)
    nc = bacc.Bacc("TRN2", target_bir_lowering=False, debug=False,
                   num_devices=NCORES)

    d_x = nc.dram_tensor("x", [1, PTS], F32, kind="ExternalInput")
    d_x16 = nc.dram_tensor("x16", [1, PTS], F16, kind="ExternalInput")
    d_g16 = nc.dram_tensor("g16", [1, 16], F16, kind="ExternalInput")
    d_wh = nc.dram_tensor("wh", [128, NHID * 4096], F16, kind="ExternalInput")
    d_wo8 = nc.dram_tensor("wo8", [128, 256], F16, kind="ExternalInput")
    d_consts = nc.dram_tensor("consts", [128, CONSTW], F32,
                              kind="ExternalInput")
    d_out = nc.dram_tensor("out", [128, OUTCOLS], F32, kind="ExternalOutput")

    with tile.TileContext(nc) as tc:
        with (
            tc.tile_pool(name="cst", bufs=1) as cst_pool,
            tc.tile_pool(name="whp", bufs=1) as wh_pool,
            tc.tile_pool(name="io", bufs=1) as io_pool,
            tc.tile_pool(name="hbuf", bufs=1) as h_pool,
            tc.tile_pool(name="vbuf", bufs=1) as v_pool,
            tc.tile_pool(name="tmp", bufs=1) as tmp_pool,
            tc.tile_pool(name="zps", bufs=1, space="PSUM") as z_ps,
            tc.tile_pool(name="yps", bufs=1, space="PSUM") as y_ps,
        ):
            t_cst = cst_pool.tile([128, CONSTW], F32, tag="consts")
            nc.sync.dma_start(t_cst[:], d_consts[:, :])
            t_x16 = io_pool.tile([1, PTS], F16, tag="x16")
            t_g16 = io_pool.tile([1, 16], F16, tag="g16")
            ap_gb = t_cst[0:8, C_GB:C_GB + 1]
            ap_bo = t_cst[0:8, C_BO:C_BO + 1]
            ap_ones8 = t_cst[0:8, C_ONES8:C_ONES8 + 1]
            ap_gw = t_g16[0:1, 0:8]
            ap_ones1 = t_cst[0:1, C_ONES1:C_ONES1 + 128]
            ap_vstd = t_cst[:, C_VSTD:C_VSTD + 1]
            ap_vwide = t_cst[:, C_VWIDE:C_VWIDE + 1]

            # hidden + output weights in fp16 (same PE rate, half the SBUF)
            t_wh = []
            for l in range(NHID):
                w = wh_pool.tile([128, 4096], F16, tag=f"wh{l}", name=f"wh{l}")
                nc.gpsimd.dma_start(w[:], d_wh[:, l * 4096:(l + 1) * 4096])
                t_wh.append(w)
            # two output-weight copies: [:, 0:128] main chunks, [:, 128:256]
            # last chunk (its SKIP_ACT->SKIP_DVE tail flips change the
            # per-source poly gain)
            t_wo8 = wh_pool.tile([128, 256], F16, tag="wo8", name="wo8")
            nc.gpsimd.dma_start(t_wo8[:], d_wo8[:, :])

            t_rso = tmp_pool.tile([128, 2 * OUTCOLS], F32, tag="rso")
            t_u = io_pool.tile([8, PTS], F32, tag="u")
            t_xc = {}

            def emit_gate_exp():
                # gate logits+exp at chunk 0's top (PE+ACT only)
                for s in range(PTS // SUB):
                    p_zg = y_ps.tile([8, SUB], F32, tag="y", name=f"zg{s}",
                                     bufs=y_bufs)
                    nc.tensor.matmul(p_zg[:], ap_gw,
                                     t_x16[:, s * SUB:(s + 1) * SUB],
                                     start=True, stop=True)
                    nc.scalar.activation(t_u[:, s * SUB:(s + 1) * SUB],
                                         p_zg[:], AT.Exp, bias=ap_gb,
                                         scale=1.0)

            def emit_gate_den():
                # denominator+reciprocal just before chunk 0's combine, so
                # the reciprocal (DVE) never blocks DVE's queue head
                p_den = z_ps.tile([128, CHUNK], F32, tag="z", name="pden",
                                  bufs=z_bufs)
                for col in range(OUTCOLS):
                    nc.tensor.matmul(p_den[:, col:col + 1],
                                     t_u[:, col * 128:(col + 1) * 128],
                                     ap_ones8, start=True, stop=True)
                nc.vector.reciprocal(t_rso[:, 0:OUTCOLS], p_den[:, 0:OUTCOLS])

            # x broadcast tiles, pipelined one chunk ahead (PSUM -> ACT copy)
            t_xb = {}

            def emit_xb(c):
                t_xc[c] = io_pool.tile([1, CHUNK], F32, tag="xc", bufs=2,
                                       name=f"xc{c}")
                nc.sync.dma_start(t_xc[c][:],
                                  d_x[0:1, c * CHUNK:(c + 1) * CHUNK])
                t = io_pool.tile([128, CHUNK], F32, tag="xb", bufs=2,
                                 name=f"xb{c}")
                if c == 0:
                    # chunk 0 via PE+ACT: Pool's GPSIMD library load (~13us)
                    # would otherwise gate the whole pipeline start
                    for s in range(NSUB):
                        p_xb = z_ps.tile([128, CHUNK], F32, tag="z",
                                         name=f"pxb{c}_{s}", bufs=z_bufs)
                        nc.tensor.matmul(p_xb[:, 0:SUB], ap_ones1,
                                         t_xc[c][:, s * SUB:(s + 1) * SUB],
                                         start=True, stop=True)
                        nc.scalar.activation(t[:, s * SUB:(s + 1) * SUB],
                                             p_xb[:, 0:SUB], AT.Identity,
                                             bias=0.0, scale=1.0)
                else:
                    nc.gpsimd.partition_broadcast(t[:], t_xc[c][:],
                                                  channels=128)
                t_xb[c] = t

            emit_xb(0)
            # gate inputs DMA'd after chunk 0's x slice so the pipeline
            # start isn't queued behind them
            nc.sync.dma_start(t_x16[:], d_x16[0:1, :])
            nc.sync.dma_start(t_g16[:], d_g16[0:1, :])

            # layer-0 software pipelining: the WRAP ops (Pool 3-op chains and
            # DVE AMW/SIN7, whose inputs are always ready) are hoisted one
            # quad ahead in their engine queues; the ACT sins stay at the
            # quad's own position (hoisting them risks head-of-line blocking
            # on the producing engine).
            slabs = {}

            def emit_l0_wrap(c, quad):
                t_h = {0: h_pool.tile([128, NB], F16, tag="hb0",
                                      bufs=h0_bufs,
                                      name=f"hb{c}_{quad}_0"),
                       1: h_pool.tile([128, NB], F16, tag="hb1",
                                      bufs=h1_bufs,
                                      name=f"hb{c}_{quad}_1")}
                t_h["l0"] = (h_pool.tile([128, NB], F16, tag="hl0",
                                         bufs=hl0_bufs,
                                         name=f"hl{c}_{quad}")
                             if hl0_bufs else t_h[0])
                slabs[(c, quad)] = t_h
                for m in range(4):
                    e = quad * 4 + m
                    for half in range(2):
                        col = e * 2 + half
                        unit = m * 2 + half
                        cls = l0_plan[col]
                        ap_a = t_cst[:, C_A0 + col:C_A0 + col + 1]
                        ap_c = t_cst[:, C_C0 + col:C_C0 + col + 1]
                        hsl = t_h["l0"][:, unit * UW:(unit + 1) * UW]
                        # wrap written straight into the fp16 slab slot, then
                        # sin in-place: the slab double-buffering is the
                        # producer runahead
                        if cls in (L0_POOL_ACT, L0_POOL_DVE):
                            t_zb = tmp_pool.tile([128, UW], F32, tag="zb",
                                                 bufs=2,
                                                 name=f"zb{c}{quad}{col}")
                            nc.gpsimd.tensor_scalar(
                                t_zb[:], t_xb[c][:], ap_a, ap_c,
                                ALU.mult, ALU.add)
                            t_k = tmp_pool.tile([128, UW], F32, tag="k",
                                                bufs=2,
                                                name=f"k{c}{quad}{col}")
                            nc.gpsimd.tensor_scalar(
                                t_k[:], t_zb[:], MAGIC, MAGIC,
                                ALU.add, ALU.subtract)
                            nc.gpsimd.tensor_tensor(
                                hsl, t_zb[:], t_k[:], ALU.subtract)
                        else:
                            nc.vector._custom_dve(
                                AMW, out=hsl, in0=t_xb[c][:],
                                s0=ap_a, s1=ap_c, imm2=MAGIC)
                        if cls in (L0_POOL_DVE, L0_DVE_DVE):
                            nc.vector._custom_dve(
                                SIN7, out=hsl, in0=hsl, in1=ap_vstd,
                                s0=0.0, s1=R_STD, imm2=U_STD)

            def emit_l0_sin(c, quad):
                t_h = slabs[(c, quad)]
                for m in range(4):
                    e = quad * 4 + m
                    for half in range(2):
                        col = e * 2 + half
                        unit = m * 2 + half
                        cls = l0_plan[col]
                        if cls in (L0_POOL_ACT, L0_DVE_ACT):
                            hsl = t_h["l0"][:, unit * UW:(unit + 1) * UW]
                            nc.scalar.activation(hsl, hsl, AT.Sin,
                                                 bias=0.0, scale=TWO_PI)

            for c in range(NCHUNK):
                if c + 1 < NCHUNK:
                    emit_xb(c + 1)
                if c == 0:
                    emit_gate_exp()

                chunk_y = [y_ps.tile([8, SUB], F32, tag="y", name=f"y{c}_{s}",
                                     bufs=y_bufs)
                           for s in range(NSUB)]

                for quad in range(2):
                    emit_l0_wrap(c, quad)
                    emit_l0_sin(c, quad)
                    t_h = slabs.pop((c, quad))

                    # ---- hidden layers, 4-expert staggered
                    for l in range(1, NLAYERS):
                        lw = l - 1
                        rpar = "l0" if l == 1 else (l - 1) & 1
                        wpar = l & 1
                        for m in range(4):
                            e = quad * 4 + m
                            for half in range(2):
                                unit = m * 2 + half
                                ti = lw * 16 + e * 2 + half
                                cls = hidden_plan[ti]
                                if c == NCHUNK - 1 and ti in tail_set:
                                    cls = SKIP_DVE
                                p_z = z_ps.tile([128, CHUNK], F32, tag="z",
                                                name=f"z{c}{quad}{l}{m}{half}",
                                                bufs=z_bufs)
                                for si in range(NSUB):
                                    for kc in range(2):
                                        wc = ((e * 2 + kc) * 2 + half) * 128
                                        ru = m * 2 + kc
                                        nc.tensor.matmul(
                                            p_z[:, si * SUB:(si + 1) * SUB],
                                            t_wh[lw][:, wc:wc + 128],
                                            t_h[rpar][:, ru * UW + si * SUB:
                                                       ru * UW + (si + 1) * SUB],
                                            start=(kc == 0), stop=(kc == 1))
                                chc = C_BH + lw * 16 + e * 2 + half
                                chr_ = C_BHR + lw * 16 + e * 2 + half
                                hsl = t_h[wpar][:, unit * UW:(unit + 1) * UW]
                                if cls in (WRAP_ACT, WRAP_DVE):
                                    t_v = v_pool.tile(
                                        [128, CHUNK], F32, tag="vh", bufs=vh_bufs,
                                        name=f"vh{c}{quad}{l}{m}{half}")
                                    nc.vector._custom_dve(
                                        AMW, out=t_v[:], in0=p_z[:],
                                        s0=1.0,
                                        s1=t_cst[:, chc:chc + 1],
                                        imm2=MAGIC)
                                    if cls == WRAP_ACT:
                                        nc.scalar.activation(
                                            hsl, t_v[:], AT.Sin,
                                            bias=0.0, scale=TWO_PI)
                                    else:
                                        nc.vector._custom_dve(
                                            SIN7, out=hsl, in0=t_v[:],
                                            in1=ap_vstd,
                                            s0=0.0, s1=R_STD, imm2=U_STD)
                                elif cls == SKIP_ACT:
                                    nc.scalar.activation(
                                        hsl, p_z[:], AT.Sin,
                                        bias=t_cst[:, chr_:chr_ + 1],
                                        scale=TWO_PI)
                                else:  # SKIP_DVE
                                    nc.vector._custom_dve(
                                        SIN7, out=hsl, in0=p_z[:],
                                        in1=ap_vwide,
                                        s0=t_cst[:, chc:chc + 1],
                                        s1=R_WIDE, imm2=U_WIDE)

                    # ---- output layer: long accumulation per subtile
                    for s in range(NSUB):
                        p_y = chunk_y[s]
                        for m in range(4):
                            e = quad * 4 + m
                            for kc in range(2):
                                ru = m * 2 + kc
                                blk = ((e * 2 + kc) * 8
                                       + (128 if c == NCHUNK - 1 else 0))
                                nc.tensor.matmul(
                                    p_y[:, :],
                                    t_wo8[:, blk:blk + 8],
                                    t_h[1][:, ru * UW + s * SUB:
                                           ru * UW + (s + 1) * SUB],
                                    start=(quad == 0 and m == 0 and kc == 0),
                                    stop=(quad == 1 and m == 3 and kc == 1),
                                    skip_group_check=True)

                if c == 0:
                    emit_gate_den()

                # ---- combine
                t_w8 = io_pool.tile([8, CHUNK], F32, tag="w8")
                for s in range(NSUB):
                    nc.vector.scalar_tensor_tensor(
                        t_w8[:, s * SUB:(s + 1) * SUB], chunk_y[s][:], ap_bo,
                        t_u[:, (c * NSUB + s) * SUB:(c * NSUB + s + 1) * SUB],
                        ALU.add, ALU.mult)
                nco = CHUNK // 128
                p_num = z_ps.tile([128, CHUNK], F32, tag="z", name=f"pnum{c}",
                                  bufs=z_bufs)
                for col in range(nco):
                    nc.tensor.matmul(p_num[:, col:col + 1],
                                     t_w8[:, col * 128:(col + 1) * 128],
                                     ap_ones8, start=True, stop=True)
                nc.vector.tensor_tensor(
                    t_rso[:, OUTCOLS + c * nco:OUTCOLS + (c + 1) * nco],
                    p_num[:, 0:nco],
                    t_rso[:, c * nco:(c + 1) * nco], ALU.mult)

            nc.sync.dma_start(d_out[:, :], t_rso[:, OUTCOLS:2 * OUTCOLS])

    nc.compile()
    return nc


def _plan_from_bounds(hb):
    """hb: [3][E][2] max |z+b| in turns (host-measured, pre-slack).
    Returns (hidden_plan, l0_plan) int tuples balancing per-chunk engine
    busy (ns units from the TRN2 cost model)."""
    flat = [hb[l][e][h] * RANGE_SLACK
            for l in range(NHID) for e in range(E) for h in range(2)]
    skip_ok = [b <= SKIP_BOUND for b in flat]
    sd_ok = [b <= SD_BOUND for b in flat]
    n_skip = sum(skip_ok)
    n_wrap = 48 - n_skip

    A_PS, A_SB, S_SB, S_PS, ACT_T, POOL3 = 1192, 1127, 1127, 1192, 1038, 5161
    FIX_DVE, FIX_ACT = 1400, 2400

    import os
    p_force = os.environ.get("PLAN_P")
    sw_force = os.environ.get("PLAN_SW")
    best = None
    MAX_SD = 4       # wide-fit (~1e-3) tiles cap
    # Pool handles only the x partition-broadcasts (keeps it on one GPSIMD
    # library, no reloads); measured best with its 3-op layer-0 path unused.
    for p in ([int(p_force)] if p_force else [0]):
        for s0 in range(17):
            for sw in ([int(sw_force)] if sw_force else range(n_wrap + 1)):
                n_sd = sum(1 for i in range(48) if skip_ok[i] and sd_ok[i])
                for sd in range(min(MAX_SD, n_sd) + 1):
                    dve = (n_wrap * A_PS + (16 - p) * A_SB
                           + (sw + s0) * S_SB + sd * S_PS + FIX_DVE)
                    act = (64 - sw - s0 - sd) * ACT_T + FIX_ACT
                    pool = p * POOL3
                    t = max(dve, act, pool)
                    err = sd * 4 + (sw + s0)  # error-stack tiebreak
                    if best is None or (t, err) < best[0]:
                        best = ((t, err), (p, s0, sw, sd))
    p, s0, sw, sd = best[1]

    # layer-0: LAST p columns on Pool (first experts stay on the fast DVE
    # path so the first hidden matmuls start early); sin7 on the last s0
    l0 = []
    for col in range(16):
        on_pool = col >= 16 - p
        use_sin7 = col < s0
        if on_pool:
            l0.append(L0_POOL_DVE if use_sin7 else L0_POOL_ACT)
        else:
            l0.append(L0_DVE_DVE if use_sin7 else L0_DVE_ACT)

    # hidden: skip tiles sorted by bound; sd smallest-bound ones (within the
    # wide poly's fit domain) on DVE-wide
    idxs = list(range(48))
    skip_idx = sorted((i for i in idxs if skip_ok[i] and sd_ok[i]),
                      key=lambda i: hb[i // 16][(i % 16) // 2][i % 2])
    sd_set = set(skip_idx[:sd])
    wrap_idx = [i for i in idxs if not skip_ok[i]]
    # spread sin7 over the wrapped tiles evenly
    sw_set = set(wrap_idx[int(round(j * len(wrap_idx) / max(sw, 1)))]
                 for j in range(sw)) if sw else set()
    hidden = []
    for i in idxs:
        if skip_ok[i]:
            hidden.append(SKIP_DVE if i in sd_set else SKIP_ACT)
        else:
            hidden.append(WRAP_DVE if i in sw_set else WRAP_ACT)

    # last-chunk tail flips: DVE drains ~20us before ACT there, so move up
    # to 8 l=3 SKIP_ACT sins onto DVE for that chunk (gain change rides the
    # second wo8 copy; wide-fit eligibility required)
    n_tf = int(os.environ.get("PLAN_TF", "0"))
    tail = [i for i in range(32, 48)
            if hidden[i] == SKIP_ACT and sd_ok[i]][:n_tf]
    return tuple(hidden), tuple(l0), tuple(tail)


LAST_RESULT = None  # BassKernelResults of the most recent run (for test.py)
LAST_PLAN = None


def kernel(x, gate_w, gate_b, w0, b0, wh, bh, wo, bo):
    x = np.asarray(x, dtype=np.float32)
    gate_w = np.asarray(gate_w, dtype=np.float32)
    gate_b = np.asarray(gate_b, dtype=np.float32)
    w0 = np.asarray(w0, dtype=np.float32)
    b0 = np.asarray(b0, dtype=np.float32)
    wh = np.asarray(wh, dtype=np.float32)
    bh = np.asarray(bh, dtype=np.float32)
    wo = np.asarray(wo, dtype=np.float32)
    bo = np.asarray(bo, dtype=np.float32)

    # host forward pass: per-(layer, expert, half) |z + b| bound (turns)
    hb = np.zeros((NHID, E, 2), dtype=np.float64)
    xf0 = x.reshape(-1)
    for lo in range(0, xf0.size, 8192):
        xs = xf0[lo:lo + 8192]
        h = np.sin(OMEGA0 * (w0[:, :, 0:1] * xs[None, None, :]
                             + b0[:, :, None])).astype(np.float32)
        for l in range(NHID):
            z = SC * (np.einsum('egh,eht->egt', wh[l], h, dtype=np.float32)
                      + bh[l][:, :, None]).astype(np.float32)
            for e in range(E):
                for hf in range(2):
                    r = float(np.abs(z[e, hf * 128:(hf + 1) * 128]).max())
                    hb[l][e][hf] = max(hb[l][e][hf], r)
            h = np.sin(TWO_PI * z).astype(np.float32)

    plan = _plan_from_bounds(hb)
    global LAST_PLAN
    LAST_PLAN = plan
    hidden_plan, l0_plan, tail_flips = plan

    # per-source-tile poly gain (h tiles computed via SIN7 hold sin/K)
    gain = np.ones((NLAYERS, E, 2), dtype=np.float64)
    for col in range(16):
        if l0_plan[col] in (L0_POOL_DVE, L0_DVE_DVE):
            gain[0][col // 2][col % 2] = K_STD
    for i in range(48):
        cls = hidden_plan[i]
        l, e, hf = i // 16, (i % 16) // 2, i % 2
        if cls == WRAP_DVE:
            gain[l + 1][e][hf] = K_STD
        elif cls == SKIP_DVE:
            gain[l + 1][e][hf] = K_WIDE

    # ---- host packing
    whp = np.zeros((128, NHID * 4096), dtype=np.float16)
    for l in range(NHID):
        for e in range(E):
            for kc in range(2):
                for mc in range(2):
                    colbase = l * 4096 + ((e * 2 + kc) * 2 + mc) * 128
                    blk = (SC * wh[l, e, mc * 128:(mc + 1) * 128,
                                   kc * 128:(kc + 1) * 128]).T  # [k, m]
                    whp[:, colbase:colbase + 128] = (
                        blk * gain[l][e][kc]).astype(np.float16)

    # second wo8 copy for the last chunk: flipped l=3 tiles there are
    # computed via the wide SIN7 poly, so their source gain becomes K_WIDE
    gain_t = gain[3].copy()
    for i in tail_flips:
        gain_t[(i - 32) // 2][i % 2] = K_WIDE
    wo8p = np.zeros((128, 256), dtype=np.float16)
    for e in range(E):
        for kc in range(2):
            wo8p[:, (e * 2 + kc) * 8 + e] = (
                wo[e, 0, kc * 128:(kc + 1) * 128] * gain[3][e][kc]
            ).astype(np.float16)
            wo8p[:, 128 + (e * 2 + kc) * 8 + e] = (
                wo[e, 0, kc * 128:(kc + 1) * 128] * gain_t[e][kc]
            ).astype(np.float16)

    consts = np.zeros((128, CONSTW), dtype=np.float32)
    for e in range(E):
        for half in range(2):
            col = e * 2 + half
            consts[:, C_A0 + col] = SC * w0[e, half * 128:(half + 1) * 128, 0]
            consts[:, C_C0 + col] = SC * b0[e, half * 128:(half + 1) * 128]
    for l in range(NHID):
        for e in range(E):
            for half in range(2):
                bcol = SC * bh[l, e, half * 128:(half + 1) * 128]
                consts[:, C_BH + l * 16 + e * 2 + half] = bcol
                consts[:, C_BHR + l * 16 + e * 2 + half] = TWO_PI * bcol
    consts[0:8, C_GB] = gate_b
    consts[0:8, C_BO] = bo[:, 0]
    consts[0:8, C_ONES8] = 1.0
    consts[:, C_VSTD] = V_STD
    consts[:, C_VWIDE] = V_WIDE
    consts[0, C_GW:C_GW + 8] = gate_w[:, 0]
    consts[0, C_ONES1:C_ONES1 + 128] = 1.0

    g16 = np.zeros((1, 16), dtype=np.float16)
    g16[0, 0:8] = gate_w[:, 0].astype(np.float16)

    xf = x.reshape(-1)                      # [B*N]
    in_maps = []
    for j in range(NCORES):
        xc = xf[j * PTS:(j + 1) * PTS].reshape(1, PTS).copy()
        in_maps.append({"x": xc, "x16": xc.astype(np.float16), "g16": g16,
                        "wh": whp, "wo8": wo8p, "consts": consts})

    if plan not in _BUILD_CACHE:
        _BUILD_CACHE[plan] = _build(plan)
    nc = _BUILD_CACHE[plan]

    global LAST_RESULT
    LAST_RESULT = run_bass_kernel_spmd(nc, in_maps, list(range(NCORES)))
    res = LAST_RESULT.results
    parts = []
    for j in range(NCORES):
        outT = res[j]["out"]                # [128, 32]
        parts.append(outT.T.reshape(-1))    # point t = col*128 + p
    full = np.concatenate(parts).astype(np.float32)
    return full.reshape(B, N, 1)


# revision 93
# speedup vs baseline: 1.5538x; 1.0058x over previous
"""MoE-SIREN (nn_MoE_36146444763329) Trainium2 Bass kernel, v2.

Dense MoE: 8 SIREN experts (1->256->256->256->256->1, sin(30*) activations),
softmax gate over experts, weighted combine. B=2, N=16384 points.

Data-parallel over 8 NeuronCores (4096 points each), no collectives.

v2 engine plan (per chunk of 1024 points; tile = [128, 1024]):
  - PE: all matmuls (hidden K=256 via 2 fp32r chunks, output M=8 padded,
    gate, x broadcast).  Angle math in "turns" (weights pre-scaled by
    omega0/2pi on host) so wrap = round-to-nearest subtraction.
  - DVE: AFFINE_MAGIC_WRAP custom op (1 pass: ym = in*s0 + s1, out =
    ym - rne(ym) via the 1.5*2^23 magic add) for both layer-0 (from SBUF
    x-broadcast) and hidden pre-activations (from PSUM); SIN7_PROD custom
    op (deg-7 odd minimax sine in factored form, poly gain folded into the
    next layer's weights) takes a slice of the sin work.
  - ACT: bulk of the sins; tiles whose |pre-activation| stays within the
    hardware Sin table's accurate range (~3.5 rad) skip the wrap and go
    PSUM -> sin directly with a per-partition bias.
  - Pool: layer-0 wrap via 3 standard ops for most (e, half) columns.
A tiny host-side balancer picks per-tile engine assignment from these
measured per-tile ranges; the result is cached per plan.
"""
import numpy as np

import concourse.bass as bass
import concourse.mybir as mybir
import concourse.tile as tile
from concourse import bacc
from concourse.bass_utils import run_bass_kernel_spmd

import concourse.dve_ops as dve_ops
from concourse.dve_ops import DveOp
from concourse.dve_spec import (Spec, Src0, C0, C1, C2, C3, lower,
                                _spill_c3_to_src1)
from concourse.dve_uop import DveOpSpec

F32 = mybir.dt.float32
F32R = mybir.dt.float32r
F16 = mybir.dt.float16
AT = mybir.ActivationFunctionType
ALU = mybir.AluOpType

B, N, E, H, NLAYERS = 2, 16384, 8, 256, 4
OMEGA0 = 30.0
NCORES = 8
PTS = B * N // NCORES            # 4096 points per core
CHUNK = 1024
NCHUNK = PTS // CHUNK            # 4
SUB = 512                        # matmul moving free dim
NSUB = CHUNK // SUB              # 2
NHID = NLAYERS - 1               # 3 hidden layers
TWO_PI = float(2.0 * np.pi)
SC = float(OMEGA0 / (2.0 * np.pi))   # pre-scale: radians -> turns
MAGIC = float(np.float32(1.5 * 2 ** 23))
OUTCOLS = PTS // 128             # 32 columns of transposed output per core
UW = CHUNK                       # one (m, half) slab width
NB = 8 * UW                      # big-tile width: 8 units

# deg-7 odd minimax sine fits, factored p(y) = K*y*(t-R)*(t^2 + U t + V),
# t = y^2.  std: |y| <= 0.5 (post-wrap), err 2.5e-4.  wide: |y| <= 0.585
# (wrap-skipped tiles incl. slack), err 9.4e-4.  K folds into the consumer
# weights host-side.
K_STD = -56.08679675286569
R_STD = 0.24996040959546928
U_STD = -1.1395020867858219
V_STD = 0.44785087867283263
K_WIDE = -49.32072447535861
R_WIDE = 0.24994924240031763
U_WIDE = -1.2688422239794543
V_WIDE = 0.5083875825756747

SKIP_BOUND = 0.65                # turns (~4.08 rad); tile range (w/ slack)
                                 # below this may use the ACT table without a
                                 # wrap (hw table tail err <= ~7e-3, and only
                                 # at a tile's few extreme-|z| points; measured
                                 # end-to-end impact is in the noise)
SD_BOUND = 0.59                  # wide SIN7 poly fit domain (turns)
RANGE_SLACK = 1.02               # host-measured range -> hw guard factor

# consts tile column layout ([128, 512] fp32)
C_A0 = 0       # 16 cols: layer0 scale SC*w0, col = e*2+half
C_C0 = 16      # 16 cols: layer0 bias  SC*b0
C_BH = 32      # 48 cols: hidden bias (turns)   (l-1)*16 + e*2 + half
C_BHR = 80     # 48 cols: hidden bias (radians, *2pi)
C_GB = 128     # 1 col: gate bias (partitions 0..7)
C_BO = 129     # 1 col: output bias (partitions 0..7)
C_ONES8 = 130  # 1 col: ones (partitions 0..7)
C_VSTD = 131   # 1 col: V_STD on all partitions
C_VWIDE = 132  # 1 col: V_WIDE on all partitions
C_GW = 136     # row 0, cols 136..143: gate weights (lhsT [1,8])
C_ONES1 = 144  # row 0, cols 144..271: ones (lhsT [1,128])
C_WO8 = 272    # 128 cols: zero-padded output lhsT blocks [(e*2+kc)*8 + e']
CONSTW = 512

# hidden tile classes
WRAP_ACT, WRAP_DVE, SKIP_ACT, SKIP_DVE = 0, 1, 2, 3
# layer-0 tile classes: (wrap engine, sin engine)
L0_POOL_ACT, L0_DVE_ACT, L0_POOL_DVE, L0_DVE_DVE = 0, 1, 2, 3

_BUILD_CACHE: dict = {}


def _register(name, spec, rd1):
    """Idempotently add a DveOp to the module-level registry; sha computed
    at runtime so the pin always matches this environment's lower()."""
    for o in dve_ops.OPS:
        if o.name == name:
            return o
    row = dve_ops._CUSTOM_DVE_ROW_BASE + len(dve_ops.OPS)
    assert row < 0x20
    dve_ops._SUB_OPCODE_FOR_NAME[name] = row
    shas = {}
    for ver in ("v3", "v4"):
        s = DveOpSpec(name=name, opcode=row, uops=lower(spec, ver=ver),
                      rd1_en=rd1)
        shas[ver] = s.sha(ver)
    op = DveOp(name, spec, subdim=False, uops_sha=shas)
    dve_ops.OPS.append(op)
    dve_ops.CUSTOM_DVE_SPECS[name] = spec
    return op


def _make_ops():
    _ym = Src0 * C0 + C1
    _amw = _ym - ((_ym + C2) - C2)

    def _amw_ref(in0, in1, s0, s1, imm2):
        ym = (in0.astype(np.float32) * np.float32(s0)).astype(np.float32)
        ym = (ym + np.float32(s1)).astype(np.float32)
        k = ((ym + np.float32(imm2)).astype(np.float32)
             - np.float32(imm2)).astype(np.float32)
        return ym - k

    amw = _register("AFFINE_MAGIC_WRAP_ANT", Spec(body=_amw, reference=_amw_ref),
                    False)

    _y2 = Src0 + C0
    _t = _y2 * _y2
    _sin = _spill_c3_to_src1(((_t - C1) * ((_t + C2) * _t + C3)) * _y2)

    def _sin_ref(in0, in1, s0, s1, imm2):
        ym = in0.astype(np.float32) + np.float32(s0)
        t = ym * ym
        v = np.asarray(in1, np.float32).reshape(-1, 1)
        return ((t - np.float32(s1)) * ((t + np.float32(imm2)) * t + v)) * ym

    sin7 = _register("SIN7_PROD_ANT", Spec(body=_sin, reference=_sin_ref), True)
    return amw, sin7


AMW, SIN7 = _make_ops()


def _build(plan, z_bufs=3, y_bufs=2, hl0_bufs=2, h0_bufs=2, h1_bufs=2,
           vh_bufs=6):
    """hl0_bufs=0 merges the layer-0 slab into the par-0 slab.
    plan = (hidden_plan, l0_plan, tail_flips): tail_flips lists l=3 tile
    indices (32..47) that run SKIP_DVE in the LAST chunk only (their gain
    change is carried by the second wo8 copy)."""
    hidden_plan, l0_plan, tail_flips = plan
    tail_set = set(tail_flips)
    nc = bacc.Bacc("TRN2", target_bir_lowering=False, debug=False,
                   num_devices=NCORES)

    d_x = nc.dram_tensor("x", [1, PTS], F32, kind="ExternalInput")
    d_x16 = nc.dram_tensor("x16", [1, PTS], F16, kind="ExternalInput")
    d_g16 = nc.dram_tensor("g16", [1, 16], F16, kind="ExternalInput")
    d_wh = nc.dram_tensor("wh", [128, NHID * 4096], F16, kind="ExternalInput")
    d_wo8 = nc.dram_tensor("wo8", [128, 256], F16, kind="ExternalInput")
    d_consts = nc.dram_tensor("consts", [128, CONSTW], F32,
                              kind="ExternalInput")
    d_out = nc.dram_tensor("out", [128, OUTCOLS], F32, kind="ExternalOutput")

    with tile.TileContext(nc) as tc:
        with (
            tc.tile_pool(name="cst", bufs=1) as cst_pool,
            tc.tile_pool(name="whp", bufs=1) as wh_pool,
            tc.tile_pool(name="io", bufs=1) as io_pool,
            tc.tile_pool(name="hbuf", bufs=1) as h_pool,
            tc.tile_pool(name="vbuf", bufs=1) as v_pool,
            tc.tile_pool(name="tmp", bufs=1) as tmp_pool,
            tc.tile_pool(name="zps", bufs=1, space="PSUM") as z_ps,
            tc.tile_pool(name="yps", bufs=1, space="PSUM") as y_ps,
        ):
            t_cst = cst_pool.tile([128, CONSTW], F32, tag="consts")
            nc.sync.dma_start(t_cst[:], d_consts[:, :])
            t_x16 = io_pool.tile([1, PTS], F16, tag="x16")
            t_g16 = io_pool.tile([1, 16], F16, tag="g16")
            ap_gb = t_cst[0:8, C_GB:C_GB + 1]
            ap_bo = t_cst[0:8, C_BO:C_BO + 1]
            ap_ones8 = t_cst[0:8, C_ONES8:C_ONES8 + 1]
            ap_gw = t_g16[0:1, 0:8]
            ap_ones1 = t_cst[0:1, C_ONES1:C_ONES1 + 128]
            ap_vstd = t_cst[:, C_VSTD:C_VSTD + 1]
            ap_vwide = t_cst[:, C_VWIDE:C_VWIDE + 1]

            # hidden + output weights in fp16 (same PE rate, half the SBUF)
            t_wh = []
            for l in range(NHID):
                w = wh_pool.tile([128, 4096], F16, tag=f"wh{l}", name=f"wh{l}")
                nc.gpsimd.dma_start(w[:], d_wh[:, l * 4096:(l + 1) * 4096])
                t_wh.append(w)
            # two output-weight copies: [:, 0:128] main chunks, [:, 128:256]
            # last chunk (its SKIP_ACT->SKIP_DVE tail flips change the
            # per-source poly gain)
            t_wo8 = wh_pool.tile([128, 256], F16, tag="wo8", name="wo8")
            nc.gpsimd.dma_start(t_wo8[:], d_wo8[:, :])

            t_rso = tmp_pool.tile([128, 2 * OUTCOLS], F32, tag="rso")
            t_u = io_pool.tile([8, PTS], F32, tag="u")
            t_xc = {}

            def emit_gate_exp():
                # gate logits+exp at chunk 0's top (PE+ACT only)
                for s in range(PTS // SUB):
                    p_zg = y_ps.tile([8, SUB], F32, tag="y", name=f"zg{s}",
                                     bufs=y_bufs)
                    nc.tensor.matmul(p_zg[:], ap_gw,
                                     t_x16[:, s * SUB:(s + 1) * SUB],
                                     start=True, stop=True)
                    nc.scalar.activation(t_u[:, s * SUB:(s + 1) * SUB],
                                         p_zg[:], AT.Exp, bias=ap_gb,
                                         scale=1.0)

            def emit_gate_den():
                # denominator+reciprocal just before chunk 0's combine, so
                # the reciprocal (DVE) never blocks DVE's queue head
                p_den = z_ps.tile([128, CHUNK], F32, tag="z", name="pden",
                                  bufs=z_bufs)
                for col in range(OUTCOLS):
                    nc.tensor.matmul(p_den[:, col:col + 1],
                                     t_u[:, col * 128:(col + 1) * 128],
                                     ap_ones8, start=True, stop=True)
                nc.vector.reciprocal(t_rso[:, 0:OUTCOLS], p_den[:, 0:OUTCOLS])

            # x broadcast tiles, pipelined one chunk ahead (PSUM -> ACT copy)
            t_xb = {}

            def emit_xb(c):
                t_xc[c] = io_pool.tile([1, CHUNK], F32, tag="xc", bufs=2,
                                       name=f"xc{c}")
                nc.sync.dma_start(t_xc[c][:],
                                  d_x[0:1, c * CHUNK:(c + 1) * CHUNK])
                t = io_pool.tile([128, CHUNK], F32, tag="xb", bufs=2,
                                 name=f"xb{c}")
                if c == 0:
                    # chunk 0 via PE+ACT: Pool's GPSIMD library load (~13us)
                    # would otherwise gate the whole pipeline start
                    for s in range(NSUB):
                        p_xb = z_ps.tile([128, CHUNK], F32, tag="z",
                                         name=f"pxb{c}_{s}", bufs=z_bufs)
                        nc.tensor.matmul(p_xb[:, 0:SUB], ap_ones1,
                                         t_xc[c][:, s * SUB:(s + 1) * SUB],
                                         start=True, stop=True)
                        nc.scalar.activation(t[:, s * SUB:(s + 1) * SUB],
                                             p_xb[:, 0:SUB], AT.Identity,
                                             bias=0.0, scale=1.0)
                else:
                    nc.gpsimd.partition_broadcast(t[:], t_xc[c][:],
                                                  channels=128)
                t_xb[c] = t

            emit_xb(0)
            # gate inputs DMA'd after chunk 0's x slice so the pipeline
            # start isn't queued behind them
            nc.sync.dma_start(t_x16[:], d_x16[0:1, :])
            nc.sync.dma_start(t_g16[:], d_g16[0:1, :])

            # layer-0 software pipelining: the WRAP ops (Pool 3-op chains and
            # DVE AMW/SIN7, whose inputs are always ready) are hoisted one
            # quad ahead in their engine queues; the ACT sins stay at the
            # quad's own position (hoisting them risks head-of-line blocking
            # on the producing engine).
            slabs = {}

            def emit_l0_wrap(c, quad):
                t_h = {0: h_pool.tile([128, NB], F16, tag="hb0",
                                      bufs=h0_bufs,
                                      name=f"hb{c}_{quad}_0"),
                       1: h_pool.tile([128, NB], F16, tag="hb1",
                                      bufs=h1_bufs,
                                      name=f"hb{c}_{quad}_1")}
                t_h["l0"] = (h_pool.tile([128, NB], F16, tag="hl0",
                                         bufs=hl0_bufs,
                                         name=f"hl{c}_{quad}")
                             if hl0_bufs else t_h[0])
                slabs[(c, quad)] = t_h
                for m in range(4):
                    e = quad * 4 + m
                    for half in range(2):
                        col = e * 2 + half
                        unit = m * 2 + half
                        cls = l0_plan[col]
                        ap_a = t_cst[:, C_A0 + col:C_A0 + col + 1]
                        ap_c = t_cst[:, C_C0 + col:C_C0 + col + 1]
                        hsl = t_h["l0"][:, unit * UW:(unit + 1) * UW]
                        # wrap written straight into the fp16 slab slot, then
                        # sin in-place: the slab double-buffering is the
                        # producer runahead
                        if cls in (L0_POOL_ACT, L0_POOL_DVE):
                            t_zb = tmp_pool.tile([128, UW], F32, tag="zb",
                                                 bufs=2,
                                                 name=f"zb{c}{quad}{col}")
                            nc.gpsimd.tensor_scalar(
                                t_zb[:], t_xb[c][:], ap_a, ap_c,
                                ALU.mult, ALU.add)
                            t_k = tmp_pool.tile([128, UW], F32, tag="k",
                                                bufs=2,
                                                name=f"k{c}{quad}{col}")
                            nc.gpsimd.tensor_scalar(
                                t_k[:], t_zb[:], MAGIC, MAGIC,
                                ALU.add, ALU.subtract)
                            nc.gpsimd.tensor_tensor(
                                hsl, t_zb[:], t_k[:], ALU.subtract)
                        else:
                            nc.vector._custom_dve(
                                AMW, out=hsl, in0=t_xb[c][:],
                                s0=ap_a, s1=ap_c, imm2=MAGIC)
                        if cls in (L0_POOL_DVE, L0_DVE_DVE):
                            nc.vector._custom_dve(
                                SIN7, out=hsl, in0=hsl, in1=ap_vstd,
                                s0=0.0, s1=R_STD, imm2=U_STD)

            def emit_l0_sin(c, quad):
                t_h = slabs[(c, quad)]
                for m in range(4):
                    e = quad * 4 + m
                    for half in range(2):
                        col = e * 2 + half
                        unit = m * 2 + half
                        cls = l0_plan[col]
                        if cls in (L0_POOL_ACT, L0_DVE_ACT):
                            hsl = t_h["l0"][:, unit * UW:(unit + 1) * UW]
                            nc.scalar.activation(hsl, hsl, AT.Sin,
                                                 bias=0.0, scale=TWO_PI)

            for c in range(NCHUNK):
                if c + 1 < NCHUNK:
                    emit_xb(c + 1)
                if c == 0:
                    emit_gate_exp()

                chunk_y = [y_ps.tile([8, SUB], F32, tag="y", name=f"y{c}_{s}",
                                     bufs=y_bufs)
                           for s in range(NSUB)]

                for quad in range(2):
                    emit_l0_wrap(c, quad)
                    emit_l0_sin(c, quad)
                    t_h = slabs.pop((c, quad))

                    # ---- hidden layers, 4-expert staggered
                    for l in range(1, NLAYERS):
                        lw = l - 1
                        rpar = "l0" if l == 1 else (l - 1) & 1
                        wpar = l & 1
                        for m in range(4):
                            e = quad * 4 + m
                            for half in range(2):
                                unit = m * 2 + half
                                ti = lw * 16 + e * 2 + half
                                cls = hidden_plan[ti]
                                if c == NCHUNK - 1 and ti in tail_set:
                                    cls = SKIP_DVE
                                p_z = z_ps.tile([128, CHUNK], F32, tag="z",
                                                name=f"z{c}{quad}{l}{m}{half}",
                                                bufs=z_bufs)
                                for si in range(NSUB):
                                    for kc in range(2):
                                        wc = ((e * 2 + kc) * 2 + half) * 128
                                        ru = m * 2 + kc
                                        nc.tensor.matmul(
                                            p_z[:, si * SUB:(si + 1) * SUB],
                                            t_wh[lw][:, wc:wc + 128],
                                            t_h[rpar][:, ru * UW + si * SUB:
                                                       ru * UW + (si + 1) * SUB],
                                            start=(kc == 0), stop=(kc == 1))
                                chc = C_BH + lw * 16 + e * 2 + half
                                chr_ = C_BHR + lw * 16 + e * 2 + half
                                hsl = t_h[wpar][:, unit * UW:(unit + 1) * UW]
                                if cls in (WRAP_ACT, WRAP_DVE):
                                    t_v = v_pool.tile(
                                        [128, CHUNK], F32, tag="vh", bufs=vh_bufs,
                                        name=f"vh{c}{quad}{l}{m}{half}")
                                    nc.vector._custom_dve(
                                        AMW, out=t_v[:], in0=p_z[:],
                                        s0=1.0,
                                        s1=t_cst[:, chc:chc + 1],
                                        imm2=MAGIC)
                                    if cls == WRAP_ACT:
                                        nc.scalar.activation(
                                            hsl, t_v[:], AT.Sin,
                                            bias=0.0, scale=TWO_PI)
                                    else:
                                        nc.vector._custom_dve(
                                            SIN7, out=hsl, in0=t_v[:],
                                            in1=ap_vstd,
                                            s0=0.0, s1=R_STD, imm2=U_STD)
                                elif cls == SKIP_ACT:
                                    nc.scalar.activation(
                                        hsl, p_z[:], AT.Sin,
                                        bias=t_cst[:, chr_:chr_ + 1],
                                        scale=TWO_PI)
                                else:  # SKIP_DVE
                                    nc.vector._custom_dve(
                                        SIN7, out=hsl, in0=p_z[:],
                                        in1=ap_vwide,
                                        s0=t_cst[:, chc:chc + 1],
                                        s1=R_WIDE, imm2=U_WIDE)

                    # ---- output layer: long accumulation per subtile
                    for s in range(NSUB):
                        p_y = chunk_y[s]
                        for m in range(4):
                            e = quad * 4 + m
                            for kc in range(2):
                                ru = m * 2 + kc
                                blk = ((e * 2 + kc) * 8
                                       + (128 if c == NCHUNK - 1 else 0))
                                nc.tensor.matmul(
                                    p_y[:, :],
                                    t_wo8[:, blk:blk + 8],
                                    t_h[1][:, ru * UW + s * SUB:
                                           ru * UW + (s + 1) * SUB],
                                    start=(quad == 0 and m == 0 and kc == 0),
                                    stop=(quad == 1 and m == 3 and kc == 1),
                                    skip_group_check=True)

                if c == 0:
                    emit_gate_den()

                # ---- combine
                t_w8 = io_pool.tile([8, CHUNK], F32, tag="w8")
                for s in range(NSUB):
                    nc.vector.scalar_tensor_tensor(
                        t_w8[:, s * SUB:(s + 1) * SUB], chunk_y[s][:], ap_bo,
                        t_u[:, (c * NSUB + s) * SUB:(c * NSUB + s + 1) * SUB],
                        ALU.add, ALU.mult)
                nco = CHUNK // 128
                p_num = z_ps.tile([128, CHUNK], F32, tag="z", name=f"pnum{c}",
                                  bufs=z_bufs)
                for col in range(nco):
                    nc.tensor.matmul(p_num[:, col:col + 1],
                                     t_w8[:, col * 128:(col + 1) * 128],
                                     ap_ones8, start=True, stop=True)
                nc.vector.tensor_tensor(
                    t_rso[:, OUTCOLS + c * nco:OUTCOLS + (c + 1) * nco],
                    p_num[:, 0:nco],
                    t_rso[:, c * nco:(c + 1) * nco], ALU.mult)

            nc.sync.dma_start(d_out[:, :], t_rso[:, OUTCOLS:2 * OUTCOLS])

    nc.compile()
    return nc


def _plan_from_bounds(hb):
    """hb: [3][E][2] max |z+b| in turns (host-measured, pre-slack).
    Returns (hidden_plan, l0_plan) int tuples balancing per-chunk engine
    busy (ns units from the TRN2 cost model)."""
    flat = [hb[l][e][h] * RANGE_SLACK
            for l in range(NHID) for e in range(E) for h in range(2)]
    skip_ok = [b <= SKIP_BOUND for b in flat]
    sd_ok = [b <= SD_BOUND for b in flat]
    n_skip = sum(skip_ok)
    n_wrap = 48 - n_skip

    A_PS, A_SB, S_SB, S_PS, ACT_T, POOL3 = 1192, 1127, 1127, 1192, 1038, 5161
    FIX_DVE, FIX_ACT = 1400, 2400

    import os
    p_force = os.environ.get("PLAN_P")
    sw_force = os.environ.get("PLAN_SW")
    best = None
    MAX_SD = 4       # wide-fit (~1e-3) tiles cap
    # Pool handles only the x partition-broadcasts (keeps it on one GPSIMD
    # library, no reloads); measured best with its 3-op layer-0 path unused.
    for p in ([int(p_force)] if p_force else [0]):
        for s0 in range(17):
            # wrapped-tile SIN7s serialize behind their own AMW on DVE;
            # schedule measures best with at most ~5 of them
            for sw in ([int(sw_force)] if sw_force
                       else range(min(n_wrap, 5) + 1)):
                n_sd = sum(1 for i in range(48) if skip_ok[i] and sd_ok[i])
                for sd in range(min(MAX_SD, n_sd) + 1):
                    dve = (n_wrap * A_PS + (16 - p) * A_SB
                           + (sw + s0) * S_SB + sd * S_PS + FIX_DVE)
                    act = (64 - sw - s0 - sd) * ACT_T + FIX_ACT
                    pool = p * POOL3
                    t = max(dve, act, pool)
                    err = sd * 4 + (sw + s0)  # error-stack tiebreak
                    if best is None or (t, err) < best[0]:
                        best = ((t, err), (p, s0, sw, sd))
    p, s0, sw, sd = best[1]

    # layer-0: LAST p columns on Pool (first experts stay on the fast DVE
    # path so the first hidden matmuls start early); sin7 on the last s0
    l0 = []
    for col in range(16):
        on_pool = col >= 16 - p
        use_sin7 = col < s0
        if on_pool:
            l0.append(L0_POOL_DVE if use_sin7 else L0_POOL_ACT)
        else:
            l0.append(L0_DVE_DVE if use_sin7 else L0_DVE_ACT)

    # hidden: skip tiles sorted by bound; sd smallest-bound ones (within the
    # wide poly's fit domain) on DVE-wide
    idxs = list(range(48))
    skip_idx = sorted((i for i in idxs if skip_ok[i] and sd_ok[i]),
                      key=lambda i: hb[i // 16][(i % 16) // 2][i % 2])
    sd_set = set(skip_idx[:sd])
    wrap_idx = [i for i in idxs if not skip_ok[i]]
    # spread sin7 over the wrapped tiles evenly
    sw_set = set(wrap_idx[int(round(j * len(wrap_idx) / max(sw, 1)))]
                 for j in range(sw)) if sw else set()
    hidden = []
    for i in idxs:
        if skip_ok[i]:
            hidden.append(SKIP_DVE if i in sd_set else SKIP_ACT)
        else:
            hidden.append(WRAP_DVE if i in sw_set else WRAP_ACT)

    # last-chunk tail flips: DVE drains ~20us before ACT there, so move up
    # to 8 l=3 SKIP_ACT sins onto DVE for that chunk (gain change rides the
    # second wo8 copy; wide-fit eligibility required)
    n_tf = int(os.environ.get("PLAN_TF", "0"))
    tail = [i for i in range(32, 48)
            if hidden[i] == SKIP_ACT and sd_ok[i]][:n_tf]
    return tuple(hidden), tuple(l0), tuple(tail)


LAST_RESULT = None  # BassKernelResults of the most recent run (for test.py)
LAST_PLAN = None


def kernel(x, gate_w, gate_b, w0, b0, wh, bh, wo, bo):
    x = np.asarray(x, dtype=np.float32)
    gate_w = np.asarray(gate_w, dtype=np.float32)
    gate_b = np.asarray(gate_b, dtype=np.float32)
    w0 = np.asarray(w0, dtype=np.float32)
    b0 = np.asarray(b0, dtype=np.float32)
    wh = np.asarray(wh, dtype=np.float32)
    bh = np.asarray(bh, dtype=np.float32)
    wo = np.asarray(wo, dtype=np.float32)
    bo = np.asarray(bo, dtype=np.float32)

    # host forward pass: per-(layer, expert, half) |z + b| bound (turns)
    hb = np.zeros((NHID, E, 2), dtype=np.float64)
    xf0 = x.reshape(-1)
    for lo in range(0, xf0.size, 8192):
        xs = xf0[lo:lo + 8192]
        h = np.sin(OMEGA0 * (w0[:, :, 0:1] * xs[None, None, :]
                             + b0[:, :, None])).astype(np.float32)
        for l in range(NHID):
            z = SC * (np.einsum('egh,eht->egt', wh[l], h, dtype=np.float32)
                      + bh[l][:, :, None]).astype(np.float32)
            for e in range(E):
                for hf in range(2):
                    r = float(np.abs(z[e, hf * 128:(hf + 1) * 128]).max())
                    hb[l][e][hf] = max(hb[l][e][hf], r)
            h = np.sin(TWO_PI * z).astype(np.float32)

    plan = _plan_from_bounds(hb)
    global LAST_PLAN
    LAST_PLAN = plan
    hidden_plan, l0_plan, tail_flips = plan

    # per-source-tile poly gain (h tiles computed via SIN7 hold sin/K)
    gain = np.ones((NLAYERS, E, 2), dtype=np.float64)
    for col in range(16):
        if l0_plan[col] in (L0_POOL_DVE, L0_DVE_DVE):
            gain[0][col // 2][col % 2] = K_STD
    for i in range(48):
        cls = hidden_plan[i]
        l, e, hf = i // 16, (i % 16) // 2, i % 2
        if cls == WRAP_DVE:
            gain[l + 1][e][hf] = K_STD
        elif cls == SKIP_DVE:
            gain[l + 1][e][hf] = K_WIDE

    # ---- host packing
    whp = np.zeros((128, NHID * 4096), dtype=np.float16)
    for l in range(NHID):
        for e in range(E):
            for kc in range(2):
                for mc in range(2):
                    colbase = l * 4096 + ((e * 2 + kc) * 2 + mc) * 128
                    blk = (SC * wh[l, e, mc * 128:(mc + 1) * 128,
                                   kc * 128:(kc + 1) * 128]).T  # [k, m]
                    whp[:, colbase:colbase + 128] = (
                        blk * gain[l][e][kc]).astype(np.float16)

    # second wo8 copy for the last chunk: flipped l=3 tiles there are
    # computed via the wide SIN7 poly, so their source gain becomes K_WIDE
    gain_t = gain[3].copy()
    for i in tail_flips:
        gain_t[(i - 32) // 2][i % 2] = K_WIDE
    wo8p = np.zeros((128, 256), dtype=np.float16)
    for e in range(E):
        for kc in range(2):
            wo8p[:, (e * 2 + kc) * 8 + e] = (
                wo[e, 0, kc * 128:(kc + 1) * 128] * gain[3][e][kc]
            ).astype(np.float16)
            wo8p[:, 128 + (e * 2 + kc) * 8 + e] = (
                wo[e, 0, kc * 128:(kc + 1) * 128] * gain_t[e][kc]
            ).astype(np.float16)

    consts = np.zeros((128, CONSTW), dtype=np.float32)
    for e in range(E):
        for half in range(2):
            col = e * 2 + half
            consts[:, C_A0 + col] = SC * w0[e, half * 128:(half + 1) * 128, 0]
            consts[:, C_C0 + col] = SC * b0[e, half * 128:(half + 1) * 128]
    for l in range(NHID):
        for e in range(E):
            for half in range(2):
                bcol = SC * bh[l, e, half * 128:(half + 1) * 128]
                consts[:, C_BH + l * 16 + e * 2 + half] = bcol
                consts[:, C_BHR + l * 16 + e * 2 + half] = TWO_PI * bcol
    consts[0:8, C_GB] = gate_b
    consts[0:8, C_BO] = bo[:, 0]
    consts[0:8, C_ONES8] = 1.0
    consts[:, C_VSTD] = V_STD
    consts[:, C_VWIDE] = V_WIDE
    consts[0, C_GW:C_GW + 8] = gate_w[:, 0]
    consts[0, C_ONES1:C_ONES1 + 128] = 1.0

    g16 = np.zeros((1, 16), dtype=np.float16)
    g16[0, 0:8] = gate_w[:, 0].astype(np.float16)

    xf = x.reshape(-1)                      # [B*N]
    in_maps = []
    for j in range(NCORES):
        xc = xf[j * PTS:(j + 1) * PTS].reshape(1, PTS).copy()
        in_maps.append({"x": xc, "x16": xc.astype(np.float16), "g16": g16,
                        "wh": whp, "wo8": wo8p, "consts": consts})

    if plan not in _BUILD_CACHE:
        _BUILD_CACHE[plan] = _build(plan)
    nc = _BUILD_CACHE[plan]

    global LAST_RESULT
    LAST_RESULT = run_bass_kernel_spmd(nc, in_maps, list(range(NCORES)))
    res = LAST_RESULT.results
    parts = []
    for j in range(NCORES):
        outT = res[j]["out"]                # [128, 32]
        parts.append(outT.T.reshape(-1))    # point t = col*128 + p
    full = np.concatenate(parts).astype(np.float32)
    return full.reshape(B, N, 1)


# revision 94
# speedup vs baseline: 1.5696x; 1.0102x over previous
"""MoE-SIREN (nn_MoE_36146444763329) Trainium2 Bass kernel, v2.

Dense MoE: 8 SIREN experts (1->256->256->256->256->1, sin(30*) activations),
softmax gate over experts, weighted combine. B=2, N=16384 points.

Data-parallel over 8 NeuronCores (4096 points each), no collectives.

v2 engine plan (per chunk of 1024 points; tile = [128, 1024]):
  - PE: all matmuls (hidden K=256 via 2 fp32r chunks, output M=8 padded,
    gate, x broadcast).  Angle math in "turns" (weights pre-scaled by
    omega0/2pi on host) so wrap = round-to-nearest subtraction.
  - DVE: AFFINE_MAGIC_WRAP custom op (1 pass: ym = in*s0 + s1, out =
    ym - rne(ym) via the 1.5*2^23 magic add) for both layer-0 (from SBUF
    x-broadcast) and hidden pre-activations (from PSUM); SIN7_PROD custom
    op (deg-7 odd minimax sine in factored form, poly gain folded into the
    next layer's weights) takes a slice of the sin work.
  - ACT: bulk of the sins; tiles whose |pre-activation| stays within the
    hardware Sin table's accurate range (~3.5 rad) skip the wrap and go
    PSUM -> sin directly with a per-partition bias.
  - Pool: layer-0 wrap via 3 standard ops for most (e, half) columns.
A tiny host-side balancer picks per-tile engine assignment from these
measured per-tile ranges; the result is cached per plan.
"""
import numpy as np

import concourse.bass as bass
import concourse.mybir as mybir
import concourse.tile as tile
from concourse import bacc
from concourse.bass_utils import run_bass_kernel_spmd

import concourse.dve_ops as dve_ops
from concourse.dve_ops import DveOp
from concourse.dve_spec import (Spec, Src0, C0, C1, C2, C3, lower,
                                _spill_c3_to_src1)
from concourse.dve_uop import DveOpSpec

F32 = mybir.dt.float32
F32R = mybir.dt.float32r
F16 = mybir.dt.float16
AT = mybir.ActivationFunctionType
ALU = mybir.AluOpType

B, N, E, H, NLAYERS = 2, 16384, 8, 256, 4
OMEGA0 = 30.0
NCORES = 8
PTS = B * N // NCORES            # 4096 points per core
CHUNK = 1024
NCHUNK = PTS // CHUNK            # 4
SUB = 512                        # matmul moving free dim
NSUB = CHUNK // SUB              # 2
NHID = NLAYERS - 1               # 3 hidden layers
TWO_PI = float(2.0 * np.pi)
SC = float(OMEGA0 / (2.0 * np.pi))   # pre-scale: radians -> turns
MAGIC = float(np.float32(1.5 * 2 ** 23))
OUTCOLS = PTS // 128             # 32 columns of transposed output per core
UW = CHUNK                       # one (m, half) slab width
NB = 8 * UW                      # big-tile width: 8 units

# deg-7 odd minimax sine fits, factored p(y) = K*y*(t-R)*(t^2 + U t + V),
# t = y^2.  std: |y| <= 0.5 (post-wrap), err 2.5e-4.  wide: |y| <= 0.585
# (wrap-skipped tiles incl. slack), err 9.4e-4.  K folds into the consumer
# weights host-side.
K_STD = -56.08679675286569
R_STD = 0.24996040959546928
U_STD = -1.1395020867858219
V_STD = 0.44785087867283263
K_WIDE = -49.32072447535861
R_WIDE = 0.24994924240031763
U_WIDE = -1.2688422239794543
V_WIDE = 0.5083875825756747

SKIP_BOUND = 0.65                # turns (~4.08 rad); tile range (w/ slack)
                                 # below this may use the ACT table without a
                                 # wrap (hw table tail err <= ~7e-3, and only
                                 # at a tile's few extreme-|z| points; measured
                                 # end-to-end impact is in the noise)
SD_BOUND = 0.59                  # wide SIN7 poly fit domain (turns)
RANGE_SLACK = 1.02               # host-measured range -> hw guard factor

# consts tile column layout ([128, 512] fp32)
C_A0 = 0       # 16 cols: layer0 scale SC*w0, col = e*2+half
C_C0 = 16      # 16 cols: layer0 bias  SC*b0
C_BH = 32      # 48 cols: hidden bias (turns)   (l-1)*16 + e*2 + half
C_BHR = 80     # 48 cols: hidden bias (radians, *2pi)
C_GB = 128     # 1 col: gate bias (partitions 0..7)
C_BO = 129     # 1 col: output bias (partitions 0..7)
C_ONES8 = 130  # 1 col: ones (partitions 0..7)
C_VSTD = 131   # 1 col: V_STD on all partitions
C_VWIDE = 132  # 1 col: V_WIDE on all partitions
C_GW = 136     # row 0, cols 136..143: gate weights (lhsT [1,8])
C_ONES1 = 144  # row 0, cols 144..271: ones (lhsT [1,128])
C_WO8 = 272    # 128 cols: zero-padded output lhsT blocks [(e*2+kc)*8 + e']
CONSTW = 512

# hidden tile classes
WRAP_ACT, WRAP_DVE, SKIP_ACT, SKIP_DVE = 0, 1, 2, 3
# layer-0 tile classes: (wrap engine, sin engine)
L0_POOL_ACT, L0_DVE_ACT, L0_POOL_DVE, L0_DVE_DVE = 0, 1, 2, 3

_BUILD_CACHE: dict = {}


def _register(name, spec, rd1):
    """Idempotently add a DveOp to the module-level registry; sha computed
    at runtime so the pin always matches this environment's lower()."""
    for o in dve_ops.OPS:
        if o.name == name:
            return o
    row = dve_ops._CUSTOM_DVE_ROW_BASE + len(dve_ops.OPS)
    assert row < 0x20
    dve_ops._SUB_OPCODE_FOR_NAME[name] = row
    shas = {}
    for ver in ("v3", "v4"):
        s = DveOpSpec(name=name, opcode=row, uops=lower(spec, ver=ver),
                      rd1_en=rd1)
        shas[ver] = s.sha(ver)
    op = DveOp(name, spec, subdim=False, uops_sha=shas)
    dve_ops.OPS.append(op)
    dve_ops.CUSTOM_DVE_SPECS[name] = spec
    return op


def _make_ops():
    _ym = Src0 * C0 + C1
    _amw = _ym - ((_ym + C2) - C2)

    def _amw_ref(in0, in1, s0, s1, imm2):
        ym = (in0.astype(np.float32) * np.float32(s0)).astype(np.float32)
        ym = (ym + np.float32(s1)).astype(np.float32)
        k = ((ym + np.float32(imm2)).astype(np.float32)
             - np.float32(imm2)).astype(np.float32)
        return ym - k

    amw = _register("AFFINE_MAGIC_WRAP_ANT", Spec(body=_amw, reference=_amw_ref),
                    False)

    _y2 = Src0 + C0
    _t = _y2 * _y2
    _sin = _spill_c3_to_src1(((_t - C1) * ((_t + C2) * _t + C3)) * _y2)

    def _sin_ref(in0, in1, s0, s1, imm2):
        ym = in0.astype(np.float32) + np.float32(s0)
        t = ym * ym
        v = np.asarray(in1, np.float32).reshape(-1, 1)
        return ((t - np.float32(s1)) * ((t + np.float32(imm2)) * t + v)) * ym

    sin7 = _register("SIN7_PROD_ANT", Spec(body=_sin, reference=_sin_ref), True)
    return amw, sin7


AMW, SIN7 = _make_ops()


def _build(plan, z_bufs=3, y_bufs=2, hl0_bufs=2, h0_bufs=2, h1_bufs=2,
           vh_bufs=6):
    """hl0_bufs=0 merges the layer-0 slab into the par-0 slab.
    plan = (hidden_plan, l0_plan, tail_flips): tail_flips lists l=3 tile
    indices (32..47) that run SKIP_DVE in the LAST chunk only (their gain
    change is carried by the second wo8 copy)."""
    hidden_plan, l0_plan, tail_flips = plan
    tail_set = set(tail_flips)
    nc = bacc.Bacc("TRN2", target_bir_lowering=False, debug=False,
                   num_devices=NCORES)

    d_x = nc.dram_tensor("x", [1, PTS], F32, kind="ExternalInput")
    d_x16 = nc.dram_tensor("x16", [1, PTS], F16, kind="ExternalInput")
    d_g16 = nc.dram_tensor("g16", [1, 16], F16, kind="ExternalInput")
    d_wh = nc.dram_tensor("wh", [128, NHID * 4096], F16, kind="ExternalInput")
    d_wo8 = nc.dram_tensor("wo8", [128, 256], F16, kind="ExternalInput")
    d_consts = nc.dram_tensor("consts", [128, CONSTW], F32,
                              kind="ExternalInput")
    d_out = nc.dram_tensor("out", [128, OUTCOLS], F32, kind="ExternalOutput")

    with tile.TileContext(nc) as tc:
        with (
            tc.tile_pool(name="cst", bufs=1) as cst_pool,
            tc.tile_pool(name="whp", bufs=1) as wh_pool,
            tc.tile_pool(name="io", bufs=1) as io_pool,
            tc.tile_pool(name="hbuf", bufs=1) as h_pool,
            tc.tile_pool(name="vbuf", bufs=1) as v_pool,
            tc.tile_pool(name="tmp", bufs=1) as tmp_pool,
            tc.tile_pool(name="zps", bufs=1, space="PSUM") as z_ps,
            tc.tile_pool(name="yps", bufs=1, space="PSUM") as y_ps,
        ):
            t_cst = cst_pool.tile([128, CONSTW], F32, tag="consts")
            nc.sync.dma_start(t_cst[:], d_consts[:, :])
            t_x16 = io_pool.tile([1, PTS], F16, tag="x16")
            t_g16 = io_pool.tile([1, 16], F16, tag="g16")
            ap_gb = t_cst[0:8, C_GB:C_GB + 1]
            ap_bo = t_cst[0:8, C_BO:C_BO + 1]
            ap_ones8 = t_cst[0:8, C_ONES8:C_ONES8 + 1]
            ap_gw = t_g16[0:1, 0:8]
            ap_ones1 = t_cst[0:1, C_ONES1:C_ONES1 + 128]
            ap_vstd = t_cst[:, C_VSTD:C_VSTD + 1]
            ap_vwide = t_cst[:, C_VWIDE:C_VWIDE + 1]

            # hidden + output weights in fp16 (same PE rate, half the SBUF)
            t_wh = []
            for l in range(NHID):
                w = wh_pool.tile([128, 4096], F16, tag=f"wh{l}", name=f"wh{l}")
                nc.gpsimd.dma_start(w[:], d_wh[:, l * 4096:(l + 1) * 4096])
                t_wh.append(w)
            # two output-weight copies: [:, 0:128] main chunks, [:, 128:256]
            # last chunk (its SKIP_ACT->SKIP_DVE tail flips change the
            # per-source poly gain)
            t_wo8 = wh_pool.tile([128, 256], F16, tag="wo8", name="wo8")
            nc.gpsimd.dma_start(t_wo8[:], d_wo8[:, :])

            t_rso = tmp_pool.tile([128, 2 * OUTCOLS], F32, tag="rso")
            t_u = io_pool.tile([8, PTS], F32, tag="u")
            t_xc = {}

            def emit_gate_exp():
                # gate logits+exp at chunk 0's top (PE+ACT only)
                for s in range(PTS // SUB):
                    p_zg = y_ps.tile([8, SUB], F32, tag="y", name=f"zg{s}",
                                     bufs=y_bufs)
                    nc.tensor.matmul(p_zg[:], ap_gw,
                                     t_x16[:, s * SUB:(s + 1) * SUB],
                                     start=True, stop=True)
                    nc.scalar.activation(t_u[:, s * SUB:(s + 1) * SUB],
                                         p_zg[:], AT.Exp, bias=ap_gb,
                                         scale=1.0)

            def emit_gate_den():
                # denominator+reciprocal just before chunk 0's combine, so
                # the reciprocal (DVE) never blocks DVE's queue head
                p_den = z_ps.tile([128, CHUNK], F32, tag="z", name="pden",
                                  bufs=z_bufs)
                for col in range(OUTCOLS):
                    nc.tensor.matmul(p_den[:, col:col + 1],
                                     t_u[:, col * 128:(col + 1) * 128],
                                     ap_ones8, start=True, stop=True)
                nc.vector.reciprocal(t_rso[:, 0:OUTCOLS], p_den[:, 0:OUTCOLS])

            # x broadcast tiles, pipelined one chunk ahead (PSUM -> ACT copy)
            t_xb = {}

            def emit_xb(c):
                t_xc[c] = io_pool.tile([1, CHUNK], F32, tag="xc", bufs=2,
                                       name=f"xc{c}")
                nc.sync.dma_start(t_xc[c][:],
                                  d_x[0:1, c * CHUNK:(c + 1) * CHUNK])
                t = io_pool.tile([128, CHUNK], F32, tag="xb", bufs=2,
                                 name=f"xb{c}")
                if c == 0:
                    # chunk 0 via PE+ACT: Pool's GPSIMD library load (~13us)
                    # would otherwise gate the whole pipeline start
                    for s in range(NSUB):
                        p_xb = z_ps.tile([128, CHUNK], F32, tag="z",
                                         name=f"pxb{c}_{s}", bufs=z_bufs)
                        nc.tensor.matmul(p_xb[:, 0:SUB], ap_ones1,
                                         t_xc[c][:, s * SUB:(s + 1) * SUB],
                                         start=True, stop=True)
                        nc.scalar.activation(t[:, s * SUB:(s + 1) * SUB],
                                             p_xb[:, 0:SUB], AT.Identity,
                                             bias=0.0, scale=1.0)
                else:
                    nc.gpsimd.partition_broadcast(t[:], t_xc[c][:],
                                                  channels=128)
                t_xb[c] = t

            emit_xb(0)
            # gate inputs DMA'd after chunk 0's x slice so the pipeline
            # start isn't queued behind them
            nc.sync.dma_start(t_x16[:], d_x16[0:1, :])
            nc.sync.dma_start(t_g16[:], d_g16[0:1, :])

            # layer-0 software pipelining: the WRAP ops (Pool 3-op chains and
            # DVE AMW/SIN7, whose inputs are always ready) are hoisted one
            # quad ahead in their engine queues; the ACT sins stay at the
            # quad's own position (hoisting them risks head-of-line blocking
            # on the producing engine).
            slabs = {}

            def emit_l0_wrap(c, quad):
                t_h = {0: h_pool.tile([128, NB], F16, tag="hb0",
                                      bufs=h0_bufs,
                                      name=f"hb{c}_{quad}_0"),
                       1: h_pool.tile([128, NB], F16, tag="hb1",
                                      bufs=h1_bufs,
                                      name=f"hb{c}_{quad}_1")}
                t_h["l0"] = (h_pool.tile([128, NB], F16, tag="hl0",
                                         bufs=hl0_bufs,
                                         name=f"hl{c}_{quad}")
                             if hl0_bufs else t_h[0])
                slabs[(c, quad)] = t_h
                for m in range(4):
                    e = quad * 4 + m
                    for half in range(2):
                        col = e * 2 + half
                        unit = m * 2 + half
                        cls = l0_plan[col]
                        ap_a = t_cst[:, C_A0 + col:C_A0 + col + 1]
                        ap_c = t_cst[:, C_C0 + col:C_C0 + col + 1]
                        hsl = t_h["l0"][:, unit * UW:(unit + 1) * UW]
                        # wrap written straight into the fp16 slab slot, then
                        # sin in-place: the slab double-buffering is the
                        # producer runahead
                        if cls in (L0_POOL_ACT, L0_POOL_DVE):
                            t_zb = tmp_pool.tile([128, UW], F32, tag="zb",
                                                 bufs=2,
                                                 name=f"zb{c}{quad}{col}")
                            nc.gpsimd.tensor_scalar(
                                t_zb[:], t_xb[c][:], ap_a, ap_c,
                                ALU.mult, ALU.add)
                            t_k = tmp_pool.tile([128, UW], F32, tag="k",
                                                bufs=2,
                                                name=f"k{c}{quad}{col}")
                            nc.gpsimd.tensor_scalar(
                                t_k[:], t_zb[:], MAGIC, MAGIC,
                                ALU.add, ALU.subtract)
                            nc.gpsimd.tensor_tensor(
                                hsl, t_zb[:], t_k[:], ALU.subtract)
                        else:
                            nc.vector._custom_dve(
                                AMW, out=hsl, in0=t_xb[c][:],
                                s0=ap_a, s1=ap_c, imm2=MAGIC)
                        if cls in (L0_POOL_DVE, L0_DVE_DVE):
                            nc.vector._custom_dve(
                                SIN7, out=hsl, in0=hsl, in1=ap_vstd,
                                s0=0.0, s1=R_STD, imm2=U_STD)

            def emit_l0_sin(c, quad):
                t_h = slabs[(c, quad)]
                for m in range(4):
                    e = quad * 4 + m
                    for half in range(2):
                        col = e * 2 + half
                        unit = m * 2 + half
                        cls = l0_plan[col]
                        if cls in (L0_POOL_ACT, L0_DVE_ACT):
                            hsl = t_h["l0"][:, unit * UW:(unit + 1) * UW]
                            nc.scalar.activation(hsl, hsl, AT.Sin,
                                                 bias=0.0, scale=TWO_PI)

            for c in range(NCHUNK):
                if c + 1 < NCHUNK:
                    emit_xb(c + 1)
                if c == 0:
                    emit_gate_exp()

                chunk_y = [y_ps.tile([8, SUB], F32, tag="y", name=f"y{c}_{s}",
                                     bufs=y_bufs)
                           for s in range(NSUB)]

                for quad in range(2):
                    emit_l0_wrap(c, quad)
                    emit_l0_sin(c, quad)
                    t_h = slabs.pop((c, quad))

                    # ---- hidden layers, 4-expert staggered
                    for l in range(1, NLAYERS):
                        lw = l - 1
                        rpar = "l0" if l == 1 else (l - 1) & 1
                        wpar = l & 1
                        for m in range(4):
                            e = quad * 4 + m
                            for half in range(2):
                                unit = m * 2 + half
                                ti = lw * 16 + e * 2 + half
                                cls = hidden_plan[ti]
                                if c == NCHUNK - 1 and ti in tail_set:
                                    cls = SKIP_DVE
                                p_z = z_ps.tile([128, CHUNK], F32, tag="z",
                                                name=f"z{c}{quad}{l}{m}{half}",
                                                bufs=z_bufs)
                                for si in range(NSUB):
                                    for kc in range(2):
                                        wc = ((e * 2 + kc) * 2 + half) * 128
                                        ru = m * 2 + kc
                                        nc.tensor.matmul(
                                            p_z[:, si * SUB:(si + 1) * SUB],
                                            t_wh[lw][:, wc:wc + 128],
                                            t_h[rpar][:, ru * UW + si * SUB:
                                                       ru * UW + (si + 1) * SUB],
                                            start=(kc == 0), stop=(kc == 1))
                                chc = C_BH + lw * 16 + e * 2 + half
                                chr_ = C_BHR + lw * 16 + e * 2 + half
                                hsl = t_h[wpar][:, unit * UW:(unit + 1) * UW]
                                if cls in (WRAP_ACT, WRAP_DVE):
                                    t_v = v_pool.tile(
                                        [128, CHUNK], F32, tag="vh", bufs=vh_bufs,
                                        name=f"vh{c}{quad}{l}{m}{half}")
                                    nc.vector._custom_dve(
                                        AMW, out=t_v[:], in0=p_z[:],
                                        s0=1.0,
                                        s1=t_cst[:, chc:chc + 1],
                                        imm2=MAGIC)
                                    if cls == WRAP_ACT:
                                        nc.scalar.activation(
                                            hsl, t_v[:], AT.Sin,
                                            bias=0.0, scale=TWO_PI)
                                    else:
                                        nc.vector._custom_dve(
                                            SIN7, out=hsl, in0=t_v[:],
                                            in1=ap_vstd,
                                            s0=0.0, s1=R_STD, imm2=U_STD)
                                elif cls == SKIP_ACT:
                                    nc.scalar.activation(
                                        hsl, p_z[:], AT.Sin,
                                        bias=t_cst[:, chr_:chr_ + 1],
                                        scale=TWO_PI)
                                else:  # SKIP_DVE
                                    nc.vector._custom_dve(
                                        SIN7, out=hsl, in0=p_z[:],
                                        in1=ap_vwide,
                                        s0=t_cst[:, chc:chc + 1],
                                        s1=R_WIDE, imm2=U_WIDE)

                    # ---- output layer: long accumulation per subtile
                    for s in range(NSUB):
                        p_y = chunk_y[s]
                        for m in range(4):
                            e = quad * 4 + m
                            for kc in range(2):
                                ru = m * 2 + kc
                                blk = ((e * 2 + kc) * 8
                                       + (128 if c == NCHUNK - 1 else 0))
                                nc.tensor.matmul(
                                    p_y[:, :],
                                    t_wo8[:, blk:blk + 8],
                                    t_h[1][:, ru * UW + s * SUB:
                                           ru * UW + (s + 1) * SUB],
                                    start=(quad == 0 and m == 0 and kc == 0),
                                    stop=(quad == 1 and m == 3 and kc == 1),
                                    skip_group_check=True)

                if c == 0:
                    emit_gate_den()

                # ---- combine
                t_w8 = io_pool.tile([8, CHUNK], F32, tag="w8")
                for s in range(NSUB):
                    nc.vector.scalar_tensor_tensor(
                        t_w8[:, s * SUB:(s + 1) * SUB], chunk_y[s][:], ap_bo,
                        t_u[:, (c * NSUB + s) * SUB:(c * NSUB + s + 1) * SUB],
                        ALU.add, ALU.mult)
                nco = CHUNK // 128
                p_num = z_ps.tile([128, CHUNK], F32, tag="z", name=f"pnum{c}",
                                  bufs=z_bufs)
                for col in range(nco):
                    nc.tensor.matmul(p_num[:, col:col + 1],
                                     t_w8[:, col * 128:(col + 1) * 128],
                                     ap_ones8, start=True, stop=True)
                nc.vector.tensor_tensor(
                    t_rso[:, OUTCOLS + c * nco:OUTCOLS + (c + 1) * nco],
                    p_num[:, 0:nco],
                    t_rso[:, c * nco:(c + 1) * nco], ALU.mult)

            nc.sync.dma_start(d_out[:, :], t_rso[:, OUTCOLS:2 * OUTCOLS])

    nc.compile()
    return nc


def _plan_from_bounds(hb):
    """hb: [3][E][2] max |z+b| in turns (host-measured, pre-slack).
    Returns (hidden_plan, l0_plan) int tuples balancing per-chunk engine
    busy (ns units from the TRN2 cost model)."""
    flat = [hb[l][e][h] * RANGE_SLACK
            for l in range(NHID) for e in range(E) for h in range(2)]
    skip_ok = [b <= SKIP_BOUND for b in flat]
    sd_ok = [b <= SD_BOUND for b in flat]
    n_skip = sum(skip_ok)
    n_wrap = 48 - n_skip

    A_PS, A_SB, S_SB, S_PS, ACT_T, POOL3 = 1192, 1127, 1127, 1192, 1038, 5161
    FIX_DVE, FIX_ACT = 1400, 2400

    import os
    p_force = os.environ.get("PLAN_P")
    sw_force = os.environ.get("PLAN_SW")
    best = None
    MAX_SD = 4       # wide-fit (~1e-3) tiles cap
    # Pool handles only the x partition-broadcasts (keeps it on one GPSIMD
    # library, no reloads); measured best with its 3-op layer-0 path unused.
    for p in ([int(p_force)] if p_force else [0]):
        for s0 in range(17):
            # wrapped-tile SIN7s serialize behind their own AMW on DVE;
            # schedule measures best with at most ~5 of them
            for sw in ([int(sw_force)] if sw_force
                       else range(min(n_wrap, 5) + 1)):
                n_sd = sum(1 for i in range(48) if skip_ok[i] and sd_ok[i])
                for sd in range(min(MAX_SD, n_sd) + 1):
                    dve = (n_wrap * A_PS + (16 - p) * A_SB
                           + (sw + s0) * S_SB + sd * S_PS + FIX_DVE)
                    act = (64 - sw - s0 - sd) * ACT_T + FIX_ACT
                    pool = p * POOL3
                    t = max(dve, act, pool)
                    err = sd * 4 + (sw + s0)  # error-stack tiebreak
                    if best is None or (t, err) < best[0]:
                        best = ((t, err), (p, s0, sw, sd))
    p, s0, sw, sd = best[1]

    # layer-0: LAST p columns on Pool (first experts stay on the fast DVE
    # path so the first hidden matmuls start early); sin7 on the last s0
    l0 = []
    for col in range(16):
        on_pool = col >= 16 - p
        # SIN7 (serial after AMW on DVE) on the LAST cols: the first experts'
        # tiles gate each quad's l=1 matmuls, so they take the pipelined
        # DVE-wrap + ACT-sin path
        use_sin7 = col >= 16 - s0 - p
        if on_pool:
            l0.append(L0_POOL_DVE if use_sin7 else L0_POOL_ACT)
        else:
            l0.append(L0_DVE_DVE if use_sin7 else L0_DVE_ACT)

    # hidden: skip tiles sorted by bound; sd smallest-bound ones (within the
    # wide poly's fit domain) on DVE-wide
    idxs = list(range(48))
    skip_idx = sorted((i for i in idxs if skip_ok[i] and sd_ok[i]),
                      key=lambda i: hb[i // 16][(i % 16) // 2][i % 2])
    sd_set = set(skip_idx[:sd])
    wrap_idx = [i for i in idxs if not skip_ok[i]]
    # spread sin7 over the wrapped tiles evenly
    sw_set = set(wrap_idx[int(round(j * len(wrap_idx) / max(sw, 1)))]
                 for j in range(sw)) if sw else set()
    hidden = []
    for i in idxs:
        if skip_ok[i]:
            hidden.append(SKIP_DVE if i in sd_set else SKIP_ACT)
        else:
            hidden.append(WRAP_DVE if i in sw_set else WRAP_ACT)

    # last-chunk tail flips: DVE drains ~20us before ACT there, so move up
    # to 8 l=3 SKIP_ACT sins onto DVE for that chunk (gain change rides the
    # second wo8 copy; wide-fit eligibility required)
    n_tf = int(os.environ.get("PLAN_TF", "0"))
    tail = [i for i in range(32, 48)
            if hidden[i] == SKIP_ACT and sd_ok[i]][:n_tf]
    return tuple(hidden), tuple(l0), tuple(tail)


LAST_RESULT = None  # BassKernelResults of the most recent run (for test.py)
LAST_PLAN = None


def kernel(x, gate_w, gate_b, w0, b0, wh, bh, wo, bo):
    x = np.asarray(x, dtype=np.float32)
    gate_w = np.asarray(gate_w, dtype=np.float32)
    gate_b = np.asarray(gate_b, dtype=np.float32)
    w0 = np.asarray(w0, dtype=np.float32)
    b0 = np.asarray(b0, dtype=np.float32)
    wh = np.asarray(wh, dtype=np.float32)
    bh = np.asarray(bh, dtype=np.float32)
    wo = np.asarray(wo, dtype=np.float32)
    bo = np.asarray(bo, dtype=np.float32)

    # host forward pass: per-(layer, expert, half) |z + b| bound (turns)
    hb = np.zeros((NHID, E, 2), dtype=np.float64)
    xf0 = x.reshape(-1)
    for lo in range(0, xf0.size, 8192):
        xs = xf0[lo:lo + 8192]
        h = np.sin(OMEGA0 * (w0[:, :, 0:1] * xs[None, None, :]
                             + b0[:, :, None])).astype(np.float32)
        for l in range(NHID):
            z = SC * (np.einsum('egh,eht->egt', wh[l], h, dtype=np.float32)
                      + bh[l][:, :, None]).astype(np.float32)
            for e in range(E):
                for hf in range(2):
                    r = float(np.abs(z[e, hf * 128:(hf + 1) * 128]).max())
                    hb[l][e][hf] = max(hb[l][e][hf], r)
            h = np.sin(TWO_PI * z).astype(np.float32)

    plan = _plan_from_bounds(hb)
    global LAST_PLAN
    LAST_PLAN = plan
    hidden_plan, l0_plan, tail_flips = plan

    # per-source-tile poly gain (h tiles computed via SIN7 hold sin/K)
    gain = np.ones((NLAYERS, E, 2), dtype=np.float64)
    for col in range(16):
        if l0_plan[col] in (L0_POOL_DVE, L0_DVE_DVE):
            gain[0][col // 2][col % 2] = K_STD
    for i in range(48):
        cls = hidden_plan[i]
        l, e, hf = i // 16, (i % 16) // 2, i % 2
        if cls == WRAP_DVE:
            gain[l + 1][e][hf] = K_STD
        elif cls == SKIP_DVE:
            gain[l + 1][e][hf] = K_WIDE

    # ---- host packing
    whp = np.zeros((128, NHID * 4096), dtype=np.float16)
    for l in range(NHID):
        for e in range(E):
            for kc in range(2):
                for mc in range(2):
                    colbase = l * 4096 + ((e * 2 + kc) * 2 + mc) * 128
                    blk = (SC * wh[l, e, mc * 128:(mc + 1) * 128,
                                   kc * 128:(kc + 1) * 128]).T  # [k, m]
                    whp[:, colbase:colbase + 128] = (
                        blk * gain[l][e][kc]).astype(np.float16)

    # second wo8 copy for the last chunk: flipped l=3 tiles there are
    # computed via the wide SIN7 poly, so their source gain becomes K_WIDE
    gain_t = gain[3].copy()
    for i in tail_flips:
        gain_t[(i - 32) // 2][i % 2] = K_WIDE
    wo8p = np.zeros((128, 256), dtype=np.float16)
    for e in range(E):
        for kc in range(2):
            wo8p[:, (e * 2 + kc) * 8 + e] = (
                wo[e, 0, kc * 128:(kc + 1) * 128] * gain[3][e][kc]
            ).astype(np.float16)
            wo8p[:, 128 + (e * 2 + kc) * 8 + e] = (
                wo[e, 0, kc * 128:(kc + 1) * 128] * gain_t[e][kc]
            ).astype(np.float16)

    consts = np.zeros((128, CONSTW), dtype=np.float32)
    for e in range(E):
        for half in range(2):
            col = e * 2 + half
            consts[:, C_A0 + col] = SC * w0[e, half * 128:(half + 1) * 128, 0]
            consts[:, C_C0 + col] = SC * b0[e, half * 128:(half + 1) * 128]
    for l in range(NHID):
        for e in range(E):
            for half in range(2):
                bcol = SC * bh[l, e, half * 128:(half + 1) * 128]
                consts[:, C_BH + l * 16 + e * 2 + half] = bcol
                consts[:, C_BHR + l * 16 + e * 2 + half] = TWO_PI * bcol
    consts[0:8, C_GB] = gate_b
    consts[0:8, C_BO] = bo[:, 0]
    consts[0:8, C_ONES8] = 1.0
    consts[:, C_VSTD] = V_STD
    consts[:, C_VWIDE] = V_WIDE
    consts[0, C_GW:C_GW + 8] = gate_w[:, 0]
    consts[0, C_ONES1:C_ONES1 + 128] = 1.0

    g16 = np.zeros((1, 16), dtype=np.float16)
    g16[0, 0:8] = gate_w[:, 0].astype(np.float16)

    xf = x.reshape(-1)                      # [B*N]
    in_maps = []
    for j in range(NCORES):
        xc = xf[j * PTS:(j + 1) * PTS].reshape(1, PTS).copy()
        in_maps.append({"x": xc, "x16": xc.astype(np.float16), "g16": g16,
                        "wh": whp, "wo8": wo8p, "consts": consts})

    if plan not in _BUILD_CACHE:
        _BUILD_CACHE[plan] = _build(plan)
    nc = _BUILD_CACHE[plan]

    global LAST_RESULT
    LAST_RESULT = run_bass_kernel_spmd(nc, in_maps, list(range(NCORES)))
    res = LAST_RESULT.results
    parts = []
    for j in range(NCORES):
        outT = res[j]["out"]                # [128, 32]
        parts.append(outT.T.reshape(-1))    # point t = col*128 + p
    full = np.concatenate(parts).astype(np.float32)
    return full.reshape(B, N, 1)
